# revision 58
# baseline (speedup 1.0000x reference)
"""Multi-head attention (B=2, S=2048, D=1024, H=16, causal mask) on 8 TRN2 cores.

Sharding: core c handles batch b = c // 4 and head-group hg = c % 4
(4 heads = 256 feature dims each). Each core computes its heads' QKV
projections, causal attention, and a partial output projection
(attn_out @ w_o[:, hg].T); the host sums the 4 partials per batch and
adds b_o.

v2 schedule: the PE p-state ramp (0.65->1.2->2.4 GHz, max only after
~3us of gap-free execution) dominates performance, so the kernel is
organized as one continuous PE stream with no cross-engine round
trips on the critical path:

  - projections are chunked by seq quarter n and software-pipelined
    with attention: proj(n) -> attn(qc=n-1 tail norms + qc=n) so the
    PE always has dense 128-contraction work between attention deps
  - exp reads score PSUM directly (no DVE staging copy); causal
    masking is a single [128,128] {0,1} lower-triangle multiply on
    the 128-col diagonal band of each diagonal block, after exp
  - softmax normalize: DVE fast-reciprocal of the PV ones-column row,
    broadcast via a 1-row f32r matmul (1 cycle/col) emitted one head
    late so the PE never waits on it, then one DVE multiply
  - output projection for q-chunk qc-1 is interleaved between the
    heads of q-chunk qc (extra always-ready PE work + spread-out
    output DMA); out is stored fp16
  - all HBM traffic on HWDGE queues, inputs chunked and enqueued from
    one engine in consumption order (the ~1.6us/descriptor enqueue
    cost staggers transfers, prioritizing the critical path)
  - the PE DVFS governor needs ~5.5us of full-array busy to reach
    2.4GHz: 16 dependency-free full-array warmup matmuls burn the
    initial DMA wait to pre-ramp it; a second burst hides the last
    head's normalize latency before the tail projection
  - attention PSUM pools close before the tail so the final output
    projection gets a 4-deep pool (no rotation stalls)

DMA discipline: this toolchain rejects DMA instructions with >1 sync
wait, and the Tile layer adds a ring-credit wait from the 3rd use of
each of the 8 HWDGE queues; _split_multi_waits hoists extra waits
onto same-engine NoOps. The general mask fallback keeps the v1 code.
"""

import sys

if "/opt/trn_rl_repo" not in sys.path:
    sys.path.insert(0, "/opt/trn_rl_repo")

import numpy as np
import ml_dtypes

BF16 = ml_dtypes.bfloat16
F16 = np.float16

B, S, D, H = 2, 2048, 1024, 16
NCORE = 8
HGROUPS = 4  # head-groups == cores per batch
HPC = H // HGROUPS  # heads per core = 4
DK = D // H  # head dim = 64
DKB = HPC * DK  # feature dims per core = 256
P = 128
QC = 512  # q chunk (one PSUM bank of fp32)
NEG = -1e9

_nc_cache = {}


def _build_causal(seq=S):
    """Fast causal-mask kernel (see module docstring)."""
    import concourse.bass as bass
    import concourse.tile as tile
    from concourse import mybir
    from contextlib import ExitStack

    f32 = mybir.dt.float32
    f16 = mybir.dt.float16
    bf16 = mybir.dt.bfloat16
    exp_fn = mybir.ActivationFunctionType.Exp
    ln_fn = mybir.ActivationFunctionType.Ln
    copy_fn = mybir.ActivationFunctionType.Copy
    nqc = seq // QC  # 4
    nkt = seq // P  # 16
    nd = D // P  # 8

    nc = bass.Bass(num_swdge_queues=1)
    xq_d = nc.dram_tensor("xq_t", [D, seq], bf16, kind="ExternalInput")
    xk_d = nc.dram_tensor("xk_t", [D, seq], bf16, kind="ExternalInput")
    xv_d = nc.dram_tensor("xv_t", [D, seq], bf16, kind="ExternalInput")
    wq_d = nc.dram_tensor("wq_p", [P, D * DKB // P], bf16, kind="ExternalInput")
    wk_d = nc.dram_tensor("wk_p", [P, D * DKB // P], bf16, kind="ExternalInput")
    wv_d = nc.dram_tensor("wv_p", [P, D * DKB // P], bf16, kind="ExternalInput")
    wo_d = nc.dram_tensor("wo_p", [P, DKB * D // P], bf16, kind="ExternalInput")
    tri_d = nc.dram_tensor("tri01", [P, P], bf16, kind="ExternalInput")
    out_d = nc.dram_tensor("out", [seq, D], f16, kind="ExternalOutput")

    with ExitStack() as ctx:
        tc = ctx.enter_context(tile.TileContext(nc))
        persist = ctx.enter_context(tc.tile_pool(name="persist", bufs=1))

        ones1 = persist.tile([1, DK], f16, tag="ones1")
        nc.vector.memset(ones1[:], 1.0)
        wq_t = persist.tile([P, D * DKB // P], bf16, tag="wq")
        wk_t = persist.tile([P, D * DKB // P], bf16, tag="wk")
        wv_t = persist.tile([P, D * DKB // P], bf16, tag="wv")
        wo_t = persist.tile([P, DKB * D // P], bf16, tag="wo")
        tri_t = persist.tile([P, P], bf16, tag="tri")
        xq_t = persist.tile([P, nd, seq], bf16, tag="xq", name="xq")
        xk_t = persist.tile([P, nd, seq], bf16, tag="xk", name="xk")
        xv_t = persist.tile([P, nd, seq], bf16, tag="xv", name="xv")

        QT, KT, AT = [], [], []
        for m in range(2):
            QT.append(persist.tile([P, seq], bf16, tag=f"qt{m}", name=f"qt{m}"))
            KT.append(persist.tile([P, seq], bf16, tag=f"kt{m}", name=f"kt{m}"))
            AT.append(persist.tile([P, seq], bf16, tag=f"at{m}", name=f"at{m}"))
        vt = [
            persist.tile([P, HPC * (DK + 1)], bf16, tag=f"v{st}", name=f"v{st}")
            for st in range(nkt)
        ]

        # ---- input DMAs: enqueue cost is ~1.6us per descriptor, so spread
        # the enqueues across engines that are idle at kernel start ----
        # Single-engine enqueue: the ~1.6us/descriptor cost staggers the
        # transfers so earlier (more critical) DMAs get the HBM bandwidth
        # first. Strict consumption order.
        def load_x_chunk(xt, xd, n):
            nc.sync.dma_start(
                out=xt[:, :, n * QC : (n + 1) * QC],
                in_=xd[:, n * QC : (n + 1) * QC].rearrange(
                    "(j p) c -> p j c", p=P
                ),
            )

        load_x_chunk(xq_t, xq_d, 0)
        nc.sync.dma_start(out=wq_t[:], in_=wq_d[:, :])
        nc.sync.dma_start(out=wk_t[:], in_=wk_d[:, :])
        load_x_chunk(xk_t, xk_d, 0)
        load_x_chunk(xv_t, xv_d, 0)
        nc.sync.dma_start(out=wv_t[:], in_=wv_d[:, :])
        nc.sync.dma_start(out=tri_t[:], in_=tri_d[:, :])
        load_x_chunk(xq_t, xq_d, 1)
        load_x_chunk(xk_t, xk_d, 1)
        load_x_chunk(xv_t, xv_d, 1)
        load_x_chunk(xq_t, xq_d, 2)
        load_x_chunk(xk_t, xk_d, 2)
        load_x_chunk(xv_t, xv_d, 2)
        load_x_chunk(xq_t, xq_d, 3)
        load_x_chunk(xk_t, xk_d, 3)
        load_x_chunk(xv_t, xv_d, 3)
        nc.sync.dma_start(out=wo_t[:], in_=wo_d[:, :])

        fp_ps = ctx.enter_context(tc.tile_pool(name="fp_ps", bufs=2, space="PSUM"))
        st_ps_cm = tc.tile_pool(name="st_ps", bufs=2, space="PSUM")
        pv_ps_cm = tc.tile_pool(name="pv_ps", bufs=2, space="PSUM")
        st_ps = st_ps_cm.__enter__()
        pv_ps = pv_ps_cm.__enter__()
        atp = ctx.enter_context(tc.tile_pool(name="atp", bufs=6))
        smallp = ctx.enter_context(tc.tile_pool(name="smallp", bufs=4))
        obp = ctx.enter_context(tc.tile_pool(name="obp", bufs=2))

        # PE p-state warm-up: the DVFS governor needs ~18us of busy time
        # before the PE reaches 2.4GHz, and short gaps don't reset it.
        # Burn the first-DMA wait (~7us, which would otherwise be idle)
        # on dependency-free matmuls so the ramp clock starts early.
        dummy = persist.tile([P, QC], bf16, tag="dummy")
        nc.vector.memset(dummy[:], 0.5)
        wup = fp_ps.tile([P, QC], f32, tag="fp", name="warmup")
        for _ in range(16):
            nc.tensor.matmul(
                wup[:], lhsT=dummy[:, 0:P], rhs=dummy[:], start=True, stop=True
            )

        pending = []  # deferred (pv, hm, hp, qc, r) normalize emissions
        ob_cur = {}  # j2 -> ob tile being assembled

        def flush_pending():
            # bcast ln(sums) via a 1-cycle/col fp16 matmul, then exp(-x) on
            # scalar recovers 1/sums at fp32; emitted one head late so the
            # PE never waits on the chain
            while pending:
                pv, hm, hp, qc, lns = pending.pop(0)
                bcp = fp_ps.tile([DK, QC], f32, tag="fp", name=f"bcp{qc}_{hp}_{hm}")
                nc.tensor.matmul(
                    bcp[:],
                    lhsT=ones1[:],
                    rhs=lns[:],
                    start=True,
                    stop=True,
                )
                bc = smallp.tile([DK, QC], f32, tag="bc", name=f"bc{qc}_{hp}_{hm}")
                nc.scalar.activation(
                    out=bc[:], in_=bcp[:], func=exp_fn, scale=-1.0
                )
                nc.vector.tensor_mul(
                    AT[hm][hp : hp + DK, qc * QC : (qc + 1) * QC],
                    pv[0:DK, :],
                    bc[:],
                )

        def proj_qk_half(xt, wt, dest, n, m, dname):
            ps = fp_ps.tile([P, QC], f32, tag="fp", name=f"ps_{dname}{m}_{n}")
            for j in range(nd):
                nc.tensor.matmul(
                    ps[:],
                    lhsT=wt[:, j * DKB + m * P : j * DKB + (m + 1) * P],
                    rhs=xt[:, j, n * QC : (n + 1) * QC],
                    start=(j == 0),
                    stop=(j == nd - 1),
                )
            nc.vector.tensor_copy(
                out=dest[m][:, n * QC : (n + 1) * QC], in_=ps[:]
            )

        def proj_qk(xt, wt, dest, n, dname):
            for m in range(2):
                proj_qk_half(xt, wt, dest, n, m, dname)

        def proj_v_tile(st):
            ps = fp_ps.tile([P, DKB], f32, tag="fp", name=f"ps_v{st}")
            for j in range(nd):
                nc.tensor.matmul(
                    ps[:],
                    lhsT=xv_t[:, j, st * P : (st + 1) * P],
                    rhs=wv_t[:, j * DKB : (j + 1) * DKB],
                    start=(j == 0),
                    stop=(j == nd - 1),
                )
            v = vt[st]
            nc.vector.memset(v[:], 1.0)
            nc.vector.tensor_copy(
                out=v[:].rearrange("p (h w) -> p h w", w=DK + 1)[:, :, 0:DK],
                in_=ps[:].rearrange("p (h w) -> p h w", w=DK),
            )

        def outproj_quarter(qc, quarter, copy_on_scalar=False, pool=None):
            j2 = 2 * qc + quarter // 2
            g = quarter % 2
            st = 2 * j2 + g
            if g == 0:
                ob_cur[j2] = obp.tile([P, 2, D], f16, tag="ob", name=f"ob{j2}")
            ob = ob_cur[j2]
            for nch in range(2):
                ps = (pool or fp_ps).tile(
                    [P, QC], f32, tag="fp", name=f"ps_o{st}_{nch}"
                )
                for m in range(2):
                    nc.tensor.matmul(
                        ps[:],
                        lhsT=AT[m][:, st * P : (st + 1) * P],
                        rhs=wo_t[:, m * D + nch * QC : m * D + (nch + 1) * QC],
                        start=(m == 0),
                        stop=(m == 1),
                    )
                dst = ob[:, g, nch * QC : (nch + 1) * QC]
                if copy_on_scalar and nch == 0:
                    nc.scalar.activation(out=dst, in_=ps[:], func=copy_fn)
                else:
                    nc.vector.tensor_copy(out=dst, in_=ps[:])
            # one store per 256-row block: descriptor enqueue costs ~1.6us
            # of serial engine time, so fewer, larger stores win. The very
            # last block stores per-half (first half fires a quarter early,
            # final transfer is half-sized) on alternating engines.
            last_j2 = copy_on_scalar and j2 == 2 * qc + 1
            if last_j2:
                eng = nc.scalar if g == 1 else nc.sync
                eng.dma_start(
                    out=out_d[st * P : (st + 1) * P, :], in_=ob[:, g, :]
                )
                if g == 1:
                    del ob_cur[j2]
            elif g == 1:
                eng = nc.scalar if (copy_on_scalar and j2 % 2 == 1) else nc.sync
                eng.dma_start(
                    out=out_d[j2 * 2 * P : (j2 + 1) * 2 * P, :].rearrange(
                        "(g p) n -> p g n", p=P
                    ),
                    in_=ob[:],
                )
                del ob_cur[j2]

        def attn_head(qc, h, filler=None, interleave=(), mid=None):
            hm, hp = divmod(h, 2)
            hp *= DK
            kts = list(range(min(nkt, (qc + 1) * (QC // P))))
            pairs = [kts[i : i + 2] for i in range(0, len(kts), 2)]
            ats = []
            for pi, pair in enumerate(pairs):
                stt = st_ps.tile(
                    [P, 2 * QC], f32, tag="st", name=f"st{qc}_{h}_{pi}"
                )
                at = atp.tile([P, 2 * QC], bf16, tag="at", name=f"a{qc}_{h}_{pi}")
                diag_any = False
                for half, kt in enumerate(pair):
                    o = kt * P - qc * QC
                    diag = o >= 0
                    oo = max(o, 0)
                    diag_any |= diag
                    nc.tensor.matmul(
                        stt[:, half * QC + oo : (half + 1) * QC],
                        lhsT=KT[hm][hp : hp + DK, kt * P : (kt + 1) * P],
                        rhs=QT[hm][hp : hp + DK, qc * QC + oo : (qc + 1) * QC],
                        start=True,
                        stop=True,
                        skip_group_check=True,
                    )
                if pi < len(interleave):
                    interleave[pi]()
                if diag_any and pair[0] == 4 * qc:
                    # first diagonal pair: one exp over the whole pair; the
                    # 128-col hole holds bounded stale scores that are
                    # never read (PV starts past it)
                    nc.scalar.activation(
                        out=at[:], in_=stt[:], func=exp_fn, scale=0.125
                    )
                    for half, kt in enumerate(pair):
                        oo = max(kt * P - qc * QC, 0)
                        nc.vector.tensor_mul(
                            at[:, half * QC + oo : half * QC + oo + P],
                            at[:, half * QC + oo : half * QC + oo + P],
                            tri_t[:],
                        )
                elif diag_any:
                    # per-kt exp spans (trimmed); mask the 128-col diagonal
                    # band with the 0/1 lower-triangle tile after exp
                    for half, kt in enumerate(pair):
                        oo = max(kt * P - qc * QC, 0)
                        nc.scalar.activation(
                            out=at[:, half * QC + oo : (half + 1) * QC],
                            in_=stt[:, half * QC + oo : (half + 1) * QC],
                            func=exp_fn,
                            scale=0.125,
                        )
                        nc.vector.tensor_mul(
                            at[:, half * QC + oo : half * QC + oo + P],
                            at[:, half * QC + oo : half * QC + oo + P],
                            tri_t[:],
                        )
                else:
                    nc.scalar.activation(
                        out=at[:], in_=stt[:], func=exp_fn, scale=0.125
                    )
                ats.append((at, pair))
            for extra in interleave[len(pairs) :]:
                extra()
            # between scores and PV: always-ready projection filler work
            # (gives exp time to land without idling the PE), then the
            # previous head's normalize broadcast, then more ready work
            # (the previous q-chunk's outproj quarter) so the first PV
            # never races its exp
            if filler is not None:
                filler()
            flush_pending()
            if mid is not None:
                mid()
            pv = pv_ps.tile([DK + 1, QC], f32, tag="pv", name=f"pv{qc}_{h}")
            last_kt = kts[-1]
            for at, pair in ats:
                for half, kt in enumerate(pair):
                    oo = max(kt * P - qc * QC, 0)
                    nc.tensor.matmul(
                        pv[:, oo:QC],
                        lhsT=vt[kt][:, h * (DK + 1) : (h + 1) * (DK + 1)],
                        rhs=at[:, half * QC + oo : (half + 1) * QC],
                        start=(kt == 0),
                        stop=(kt == last_kt),
                        skip_group_check=True,
                    )
            lns = smallp.tile([1, QC], f16, tag="lns", name=f"lns{qc}_{h}")
            nc.scalar.activation(
                out=lns[:], in_=pv[DK : DK + 1, :], func=ln_fn
            )
            pending.append((pv, hm, hp, qc, lns))

        def attn(qc, fillers, pre=(), defer_last_quarter=False):
            for h in range(HPC):
                # h0's quarter reads AT rows written by this head's flush,
                # so it must trail the PVs; later heads' quarters are a
                # q-chunk old and slot in before the PVs as extra runway
                mid = None
                if qc > 0 and 0 < h and not (defer_last_quarter and h == HPC - 1):
                    mid = lambda h=h: outproj_quarter(qc - 1, h)
                attn_head(
                    qc,
                    h,
                    fillers[h] if h < len(fillers) else None,
                    interleave=pre if h == 0 else (),
                    mid=mid,
                )
                if qc > 0 and h == 0:
                    outproj_quarter(qc - 1, 0)
            if qc > 0 and defer_last_quarter:
                # always-ready PE work covering the last head's ln(sum)
                # latency before the final flush
                outproj_quarter(qc - 1, HPC - 1)

        def qk_fillers(n):
            return [
                lambda m=m, xt=xt, wt=wt, dst=dst, nm=nm: proj_qk_half(
                    xt, wt, dst, n, m, nm
                )
                for xt, wt, dst, nm in ((xq_t, wq_t, QT, "q"), (xk_t, wk_t, KT, "k"))
                for m in range(2)
            ]

        proj_qk(xq_t, wq_t, QT, 0, "q")
        proj_qk(xk_t, wk_t, KT, 0, "k")
        for n in range(nqc):
            pre = [
                (lambda st=st: proj_v_tile(st))
                for st in range(4 * n, 4 * n + 4)
            ]
            attn(
                n,
                qk_fillers(n + 1) if n + 1 < nqc else [],
                pre=pre,
                defer_last_quarter=(n + 1 == nqc),
            )
        flush_pending()
        # attention PSUM pools are done; reuse their banks for a deeper
        # tail pool so the final output projection streams without
        # rotation stalls
        pv_ps_cm.__exit__(None, None, None)
        st_ps_cm.__exit__(None, None, None)
        with tc.tile_pool(name="tailp", bufs=4, space="PSUM") as tailp:
            # dependency-free burst hides the last head's normalize (DVE
            # mul) latency before the tail projection reads AT
            wup2 = tailp.tile([P, QC], f32, tag="fp", name="tailwarm")
            for _ in range(8):
                nc.tensor.matmul(
                    wup2[:],
                    lhsT=dummy[:, 0:P],
                    rhs=dummy[:],
                    start=True,
                    stop=True,
                )
            for q in range(4):
                outproj_quarter(nqc - 1, q, copy_on_scalar=True, pool=tailp)

    return nc


def _build(mask_mode, seq=S):
    """v1 builder kept for the 'none'/'full' mask fallbacks."""
    import concourse.bass as bass
    import concourse.tile as tile
    from concourse import mybir
    from contextlib import ExitStack

    f32 = mybir.dt.float32
    bf16 = mybir.dt.bfloat16
    nqc = seq // QC
    nkt = seq // P
    nd = D // P  # 8 d-chunks

    nc = bass.Bass(num_swdge_queues=4)
    xq_d = nc.dram_tensor("xq_t", [D, seq], bf16, kind="ExternalInput")
    xk_d = nc.dram_tensor("xk_t", [D, seq], bf16, kind="ExternalInput")
    xv_d = nc.dram_tensor("xv_t", [D, seq], bf16, kind="ExternalInput")
    wq_d = nc.dram_tensor("wq_p", [P, D * DKB // P], bf16, kind="ExternalInput")
    wk_d = nc.dram_tensor("wk_p", [P, D * DKB // P], bf16, kind="ExternalInput")
    wv_d = nc.dram_tensor("wv_p", [P, D * DKB // P], bf16, kind="ExternalInput")
    wo_d = nc.dram_tensor("wo_p", [P, DKB * D // P], bf16, kind="ExternalInput")
    if mask_mode == "causal":
        stair_d = nc.dram_tensor("stair", [P, QC + 384], bf16, kind="ExternalInput")
    if mask_mode == "full":
        maskt_d = nc.dram_tensor("mask_t", [seq, seq], bf16, kind="ExternalInput")
    out_d = nc.dram_tensor("out", [seq, D], f32, kind="ExternalOutput")

    with ExitStack() as ctx:
        tc = ctx.enter_context(tile.TileContext(nc))
        persist = ctx.enter_context(tc.tile_pool(name="persist", bufs=1))

        ones64 = persist.tile([1, DK], f32, tag="ones64")
        nc.vector.memset(ones64[:], 1.0)
        wq_t = persist.tile([P, D * DKB // P], bf16, tag="wq")
        wk_t = persist.tile([P, D * DKB // P], bf16, tag="wk")
        wv_t = persist.tile([P, D * DKB // P], bf16, tag="wv")
        wo_t = persist.tile([P, DKB * D // P], bf16, tag="wo")
        nc.gpsimd.dma_start(out=wq_t[:], in_=wq_d[:, :])
        nc.gpsimd.dma_start(out=wk_t[:], in_=wk_d[:, :])
        nc.gpsimd.dma_start(out=wv_t[:], in_=wv_d[:, :])
        nc.gpsimd.dma_start(out=wo_t[:], in_=wo_d[:, :])
        if mask_mode == "causal":
            stair_t = persist.tile([P, QC + 384], bf16, tag="stair")
            nc.gpsimd.dma_start(out=stair_t[:], in_=stair_d[:, :])

        QT, KT, vt = [], [], []
        for m in range(2):
            QT.append(persist.tile([P, seq], bf16, tag=f"qt{m}", name=f"qt{m}"))
            KT.append(persist.tile([P, seq], bf16, tag=f"kt{m}", name=f"kt{m}"))
        AT = []
        for m in range(2):
            AT.append(persist.tile([P, seq], bf16, tag=f"at{m}", name=f"at{m}"))

        # ---- phase 1: projections (own PSUM + x pools, released after) ----
        with tc.tile_pool(name="xpool", bufs=1) as xpool, tc.tile_pool(
            name="projp", bufs=2, space="PSUM"
        ) as projp:

            def load_xt(xdram, name):
                t = xpool.tile([P, nd, seq], bf16, tag=name, name=name)
                h = nd // 2
                nc.sync.dma_start(
                    out=t[:, 0:h, :],
                    in_=xdram[: h * P, :].rearrange("(j p) s -> p j s", p=P),
                )
                nc.sync.dma_start(
                    out=t[:, h:nd, :],
                    in_=xdram[h * P :, :].rearrange("(j p) s -> p j s", p=P),
                )
                return t

            xq_t = load_xt(xq_d, "xq")
            xk_t = load_xt(xk_d, "xk")
            xv_t = load_xt(xv_d, "xv")

            def project_T(xt, wtile, res, name):
                ngroups = [
                    list(range(i, min(i + 2, nqc))) for i in range(0, nqc, 2)
                ]
                for m in range(2):
                    for gi, grp in enumerate(ngroups):
                        ps = projp.tile(
                            [P, len(grp) * QC],
                            f32,
                            tag="pj",
                            name=f"ps_{name}{m}_{gi}",
                        )
                        for half, n in enumerate(grp):
                            for j in range(nd):
                                nc.tensor.matmul(
                                    ps[:, half * QC : (half + 1) * QC],
                                    lhsT=wtile[
                                        :, j * DKB + m * P : j * DKB + (m + 1) * P
                                    ],
                                    rhs=xt[:, j, n * QC : (n + 1) * QC],
                                    start=(j == 0),
                                    stop=(j == nd - 1),
                                )
                        nc.vector.tensor_copy(
                            out=res[m][:, grp[0] * QC : (grp[-1] + 1) * QC],
                            in_=ps[:],
                        )

            project_T(xq_t, wq_t, QT, "qt")
            project_T(xk_t, wk_t, KT, "kt")

            # V natural layout [s, dv] + ones column per head
            for st in range(nkt):
                ps = projp.tile([P, DKB], f32, tag="pj", name=f"ps_v{st}")
                for j in range(nd):
                    nc.tensor.matmul(
                        ps[:],
                        lhsT=xv_t[:, j, st * P : (st + 1) * P],
                        rhs=wv_t[:, j * DKB : (j + 1) * DKB],
                        start=(j == 0),
                        stop=(j == nd - 1),
                    )
                v = persist.tile(
                    [P, HPC * (DK + 1)], bf16, tag=f"v{st}", name=f"v{st}"
                )
                nc.vector.memset(v[:], 1.0)
                nc.vector.tensor_copy(
                    out=v[:].rearrange("p (h w) -> p h w", w=DK + 1)[:, :, 0:DK],
                    in_=ps[:].rearrange("p (h w) -> p h w", w=DK),
                )
                vt.append(v)

        # ---- phase 2: attention (+ per-qc output projection) ----
        st_ps = ctx.enter_context(tc.tile_pool(name="st_ps", bufs=4, space="PSUM"))
        pv_ps = ctx.enter_context(tc.tile_pool(name="pv_ps", bufs=2, space="PSUM"))
        fp_ps = ctx.enter_context(tc.tile_pool(name="fp_ps", bufs=2, space="PSUM"))
        sc_pool = ctx.enter_context(tc.tile_pool(name="sc_pool", bufs=8))
        attn_pool = ctx.enter_context(tc.tile_pool(name="attn_pool", bufs=8))
        small = ctx.enter_context(tc.tile_pool(name="small", bufs=2))
        outp = ctx.enter_context(tc.tile_pool(name="outp", bufs=2))
        maskp = None
        if mask_mode == "full":
            maskp = ctx.enter_context(tc.tile_pool(name="maskp", bufs=2))

        exp_fn = mybir.ActivationFunctionType.Exp
        ln_fn = mybir.ActivationFunctionType.Ln
        for qc in range(nqc):
            mt = None
            if mask_mode == "full":
                mt = maskp.tile([P, nkt, QC], bf16, tag="mask", name=f"mt{qc}")
                nc.gpsimd.dma_start(
                    out=mt[:],
                    in_=maskt_d[:, qc * QC : (qc + 1) * QC].rearrange(
                        "(kt p) c -> p kt c", p=P
                    ),
                )
            for h in range(HPC):
                hm, hp = divmod(h, 2)
                hp *= DK
                if mask_mode == "causal":
                    kts = list(range(min(nkt, (qc + 1) * (QC // P))))
                else:
                    kts = list(range(nkt))
                pairs = [kts[i : i + 2] for i in range(0, len(kts), 2)]

                def trim(kt):
                    if mask_mode != "causal":
                        return 0, False
                    o = kt * P - qc * QC
                    return (o, True) if o >= 0 else (0, False)

                pv = pv_ps.tile([DK + 1, QC], f32, tag="pv", name=f"pv{qc}_{h}")

                def emit_pv(at, pair, is_last):
                    for half, kt in enumerate(pair):
                        o, _ = trim(kt)
                        nc.tensor.matmul(
                            pv[:, o:QC],
                            lhsT=vt[kt][:, h * (DK + 1) : (h + 1) * (DK + 1)],
                            rhs=at[:, half * QC + o : (half + 1) * QC],
                            start=(kt == 0),
                            stop=(is_last and half == len(pair) - 1),
                            skip_group_check=True,
                        )

                ats = []
                for pi, pair in enumerate(pairs):
                    sc = sc_pool.tile(
                        [P, 2 * QC], f32, tag="sc", name=f"sc{qc}_{h}_{pi}"
                    )
                    for half, kt in enumerate(pair):
                        o, diag = trim(kt)
                        stt = st_ps.tile(
                            [P, QC], f32, tag="st", name=f"st{qc}_{h}_{kt}"
                        )
                        nc.tensor.matmul(
                            stt[:, o:QC],
                            lhsT=KT[hm][hp : hp + DK, kt * P : (kt + 1) * P],
                            rhs=QT[hm][
                                hp : hp + DK, qc * QC + o : (qc + 1) * QC
                            ],
                            start=True,
                            stop=True,
                        )
                        dst = sc[:, half * QC + o : (half + 1) * QC]
                        if diag:
                            nc.vector.tensor_add(
                                out=dst,
                                in0=stt[:, o:QC],
                                in1=stair_t[:, 384 : 384 + QC - o],
                            )
                        elif mask_mode == "full":
                            nc.vector.tensor_add(
                                out=dst, in0=stt[:, o:QC], in1=mt[:, kt, :]
                            )
                        else:
                            nc.vector.tensor_copy(out=dst, in_=stt[:, o:QC])
                    at = attn_pool.tile(
                        [P, 2 * QC], bf16, tag="attn", name=f"a{qc}_{h}_{pi}"
                    )
                    o0, _ = trim(pair[0])
                    o1 = trim(pair[1])[0] if len(pair) > 1 else 0
                    if o1 == 0:
                        spans = [(o0, len(pair) * QC)]
                    else:
                        spans = [(o0, QC), (QC + o1, 2 * QC)]
                    for lo, hi in spans:
                        nc.scalar.activation(
                            out=at[:, lo:hi],
                            in_=sc[:, lo:hi],
                            func=exp_fn,
                            scale=0.125,
                        )
                    ats.append((at, pair))
                for at, pair in ats:
                    emit_pv(at, pair, pair is pairs[-1])
                # normalize: 1/sums = exp(-ln(sums)), broadcast via PE
                lns = small.tile([1, QC], f32, tag="lns", name=f"ln{qc}_{h}")
                nc.scalar.activation(
                    out=lns[:], in_=pv[DK : DK + 1, :], func=ln_fn
                )
                bcp = fp_ps.tile([DK, QC], f32, tag="fp", name=f"bcp{qc}_{h}")
                nc.tensor.matmul(
                    bcp[:], lhsT=ones64[:], rhs=lns[:], start=True, stop=True
                )
                bc = small.tile([DK, QC], f32, tag="bcast", name=f"bc{qc}_{h}")
                nc.scalar.activation(
                    out=bc[:], in_=bcp[:], func=exp_fn, scale=-1.0
                )
                nc.vector.tensor_mul(
                    AT[hm][hp : hp + DK, qc * QC : (qc + 1) * QC],
                    pv[0:DK, :],
                    bc[:],
                )

            # ---- output projection for this qc's s-tiles (keeps PE on
            # full-array matmuls between attention chunks; removes tail) ----
            for j2 in range(qc * (QC // (2 * P)), (qc + 1) * (QC // (2 * P))):
                ob = outp.tile([P, 2, D], f32, tag="ob", name=f"ob{j2}")
                for g in range(2):
                    st = 2 * j2 + g
                    for nch in range(D // QC):
                        ps = fp_ps.tile(
                            [P, QC], f32, tag="fp", name=f"ps_o{st}_{nch}"
                        )
                        for m in range(2):
                            nc.tensor.matmul(
                                ps[:],
                                lhsT=AT[m][:, st * P : (st + 1) * P],
                                rhs=wo_t[
                                    :, m * D + nch * QC : m * D + (nch + 1) * QC
                                ],
                                start=(m == 0),
                                stop=(m == 1),
                            )
                        nc.vector.tensor_copy(
                            out=ob[:, g, nch * QC : (nch + 1) * QC], in_=ps[:]
                        )
                nc.sync.dma_start(
                    out=out_d[j2 * 2 * P : (j2 + 1) * 2 * P, :].rearrange(
                        "(g p) n -> p g n", p=P
                    ),
                    in_=ob[:],
                )

    return nc


def _split_multi_waits(nc):
    """This toolchain's walrus accepts at most one sync-wait per
    instruction. Hoist extra waits onto preceding same-engine NoOps —
    engine streams execute in order, so a NoOp that blocks on a
    semaphore gates everything after it (including HWDGE descriptor
    enqueues, which happen when the issuing engine's sequencer reaches
    the DMA instruction)."""
    import bass_rust

    ctr = 0
    for f in nc.m.functions:
        for bb in f.blocks:
            insts = bb.instructions
            new = []
            changed = False
            for inst in insts:
                si = inst.sync_info
                if si is not None and len(si.on_wait) > 1:
                    waits = list(si.on_wait)
                    for w in waits[:-1]:
                        ctr += 1
                        nop = bass_rust.InstNoOp(
                            name=f"wsplit_{ctr}", ins=[], outs=[]
                        )
                        nop.engine = inst.engine
                        nop.sync_info = bass_rust.SyncInfo(
                            on_wait=[w], on_update=[]
                        )
                        new.append(nop)
                    inst.sync_info = bass_rust.SyncInfo(
                        on_wait=[waits[-1]], on_update=list(si.on_update)
                    )
                    changed = True
                new.append(inst)
            if changed:
                try:
                    bb.instructions = new
                except AttributeError:
                    insts.clear()
                    insts.extend(new)
    return nc


def _get_nc(mask_mode, seq=S, split_waits=True):
    key = (mask_mode, seq, split_waits)
    if key not in _nc_cache:
        if mask_mode == "causal":
            nc = _build_causal(seq)
        else:
            nc = _build(mask_mode, seq)
        if split_waits:
            _split_multi_waits(nc)
        _nc_cache[key] = nc
    return _nc_cache[key]


def _pack_w(w_slice_T, ncols):
    # [D_in, ncols] -> [128, D_in/128 * ncols]: col block j holds rows j*128..
    d_in = w_slice_T.shape[0]
    return (
        w_slice_T.reshape(d_in // P, P, ncols).transpose(1, 0, 2).reshape(P, -1)
    )


def _stair_np():
    cc = np.arange(QC + 384)[None, :]
    r = np.arange(P)[:, None]
    return np.where(cc < r + 384, np.float32(NEG), np.float32(0.0)).astype(BF16)


def _tri_np():
    # 0 where col < row (strictly below the shifted diagonal = masked)
    c = np.arange(P)[None, :]
    r = np.arange(P)[:, None]
    return np.where(c < r, np.float32(0.0), np.float32(1.0)).astype(BF16)


def _detect_mask_mode(mask):
    if not mask.any():
        return "none"
    causal = np.triu(np.ones((mask.shape[1], mask.shape[2]), bool), k=1)
    if all(np.array_equal(mask[b], causal) for b in range(mask.shape[0])):
        return "causal"
    return "full"


def _make_in_maps(query, key, value, mask, w_q, w_k, w_v, w_o, mask_mode, seq=S):
    per_batch = []
    for b in range(B):
        d = {
            "xq_t": np.ascontiguousarray(query[b].T).astype(BF16),
            "xk_t": np.ascontiguousarray(key[b].T).astype(BF16),
            "xv_t": np.ascontiguousarray(value[b].T).astype(BF16),
        }
        if mask_mode == "full":
            d["mask_t"] = np.where(
                mask[b].T, np.float32(NEG), np.float32(0.0)
            ).astype(BF16)
        per_batch.append(d)
    per_hg = []
    for hg in range(HGROUPS):
        rows = slice(hg * DKB, (hg + 1) * DKB)
        per_hg.append(
            {
                "wq_p": _pack_w(w_q[rows, :].T.astype(BF16), DKB),
                "wk_p": _pack_w(w_k[rows, :].T.astype(BF16), DKB),
                "wv_p": _pack_w(w_v[rows, :].T.astype(BF16), DKB),
                "wo_p": _pack_w(w_o[:, rows].T.astype(BF16), D),
            }
        )
    if mask_mode == "causal":
        extra = {"tri01": _tri_np()}
    elif mask_mode == "none":
        extra = {}
    else:
        extra = {}
    in_maps = []
    for c in range(NCORE):
        b, hg = divmod(c, HGROUPS)
        im = dict(per_batch[b])
        im.update(per_hg[hg])
        im.update(extra)
        in_maps.append(im)
    return in_maps


def _run(inputs, trace=False):
    from concourse.bass_utils import run_bass_kernel_spmd

    query = np.asarray(inputs["query"], np.float32)
    key = np.asarray(inputs["key"], np.float32)
    value = np.asarray(inputs["value"], np.float32)
    mask = np.asarray(inputs["mask"], bool)
    w_q = np.asarray(inputs["w_q"], np.float32)
    w_k = np.asarray(inputs["w_k"], np.float32)
    w_v = np.asarray(inputs["w_v"], np.float32)
    w_o = np.asarray(inputs["w_o"], np.float32)
    b_o = np.asarray(inputs["b_o"], np.float32)
    assert query.shape == (B, S, D), query.shape

    mask_mode = _detect_mask_mode(mask)
    nc = _get_nc(mask_mode)
    in_maps = _make_in_maps(query, key, value, mask, w_q, w_k, w_v, w_o, mask_mode)
    res = run_bass_kernel_spmd(nc, in_maps, list(range(NCORE)), trace=trace)
    outs = [np.asarray(r["out"], np.float32) for r in res.results]
    full = np.empty((B, S, D), np.float32)
    for b in range(B):
        full[b] = outs[HGROUPS * b]
        for i in range(1, HGROUPS):
            full[b] += outs[HGROUPS * b + i]
    full += b_o[None, None, :]
    return full, res


def kernel(**inputs):
    out, _ = _run(inputs, trace=False)
    return out


if __name__ == "__main__":
    import tempfile
    from concourse.bass_utils import compile_bass_kernel

    mode = sys.argv[1] if len(sys.argv) > 1 else "causal"
    nc = _get_nc(mode)
    from collections import Counter

    c = Counter()
    for name, inst in nc.inst_map.items():
        if "DMACopy" in type(inst).__name__:
            c[str(inst).count("wait:")] += 1
    print("DMA wait dist:", dict(c))
    td = tempfile.mkdtemp()
    p = compile_bass_kernel(nc, td)
    print("COMPILED OK:", p)


# revision 59
# speedup vs baseline: 1.1697x; 1.1697x over previous
"""Multi-head attention (B=2, S=2048, D=1024, H=16, causal mask) on 8 TRN2 cores.

Sharding: core c handles batch b = c // 4 and head-group hg = c % 4
(4 heads = 256 feature dims each). Each core computes its heads' QKV
projections, causal attention, and a partial output projection
(attn_out @ w_o[:, hg].T); the host sums the 4 partials per batch and
adds b_o.

v2 schedule: the PE p-state ramp (0.65->1.2->2.4 GHz, max only after
~3us of gap-free execution) dominates performance, so the kernel is
organized as one continuous PE stream with no cross-engine round
trips on the critical path:

  - projections are chunked by seq quarter n and software-pipelined
    with attention: proj(n) -> attn(qc=n-1 tail norms + qc=n) so the
    PE always has dense 128-contraction work between attention deps
  - exp reads score PSUM directly (no DVE staging copy); causal
    masking is a single [128,128] {0,1} lower-triangle multiply on
    the 128-col diagonal band of each diagonal block, after exp
  - softmax normalize: DVE fast-reciprocal of the PV ones-column row,
    broadcast via a 1-row f32r matmul (1 cycle/col) emitted one head
    late so the PE never waits on it, then one DVE multiply
  - output projection for q-chunk qc-1 is interleaved between the
    heads of q-chunk qc (extra always-ready PE work + spread-out
    output DMA); out is stored fp16
  - all HBM traffic on HWDGE queues, inputs chunked and enqueued from
    one engine in consumption order (the ~1.6us/descriptor enqueue
    cost staggers transfers, prioritizing the critical path)
  - the PE DVFS governor needs ~5.5us of full-array busy to reach
    2.4GHz: 16 dependency-free full-array warmup matmuls burn the
    initial DMA wait to pre-ramp it; a second burst hides the last
    head's normalize latency before the tail projection
  - attention PSUM pools close before the tail so the final output
    projection gets a 4-deep pool (no rotation stalls)

DMA discipline: this toolchain rejects DMA instructions with >1 sync
wait, and the Tile layer adds a ring-credit wait from the 3rd use of
each of the 8 HWDGE queues; _split_multi_waits hoists extra waits
onto same-engine NoOps. The general mask fallback keeps the v1 code.
"""

import sys

if "/opt/trn_rl_repo" not in sys.path:
    sys.path.insert(0, "/opt/trn_rl_repo")

import numpy as np
import ml_dtypes

BF16 = ml_dtypes.bfloat16
F16 = np.float16

B, S, D, H = 2, 2048, 1024, 16
NCORE = 8
HGROUPS = 4  # head-groups == cores per batch
HPC = H // HGROUPS  # heads per core = 4
DK = D // H  # head dim = 64
DKB = HPC * DK  # feature dims per core = 256
P = 128
QC = 512  # q chunk (one PSUM bank of fp32)
NEG = -1e9

_nc_cache = {}


def _build_causal(seq=S):
    """Fast causal-mask kernel (see module docstring)."""
    import concourse.bass as bass
    import concourse.tile as tile
    from concourse import mybir
    from contextlib import ExitStack

    f32 = mybir.dt.float32
    f16 = mybir.dt.float16
    bf16 = mybir.dt.bfloat16
    exp_fn = mybir.ActivationFunctionType.Exp
    ln_fn = mybir.ActivationFunctionType.Ln
    copy_fn = mybir.ActivationFunctionType.Copy
    nqc = seq // QC  # 4
    nkt = seq // P  # 16
    nd = D // P  # 8

    nc = bass.Bass(num_swdge_queues=1)
    xq_d = nc.dram_tensor("xq_t", [D, seq], bf16, kind="ExternalInput")
    xk_d = nc.dram_tensor("xk_t", [D, seq], bf16, kind="ExternalInput")
    xv_d = nc.dram_tensor("xv_t", [D, seq], bf16, kind="ExternalInput")
    wq_d = nc.dram_tensor("wq_p", [P, D * DKB // P], bf16, kind="ExternalInput")
    wk_d = nc.dram_tensor("wk_p", [P, D * DKB // P], bf16, kind="ExternalInput")
    wv_d = nc.dram_tensor("wv_p", [P, D * DKB // P], bf16, kind="ExternalInput")
    wo_d = nc.dram_tensor("wo_p", [P, DKB * D // P], bf16, kind="ExternalInput")
    tri_d = nc.dram_tensor("tri01", [P, P], bf16, kind="ExternalInput")
    out_d = nc.dram_tensor("out", [seq, D], f16, kind="ExternalOutput")

    with ExitStack() as ctx:
        tc = ctx.enter_context(tile.TileContext(nc))
        persist = ctx.enter_context(tc.tile_pool(name="persist", bufs=1))

        ones1 = persist.tile([1, DK], f16, tag="ones1")
        nc.vector.memset(ones1[:], 1.0)
        wq_t = persist.tile([P, D * DKB // P], bf16, tag="wq")
        wk_t = persist.tile([P, D * DKB // P], bf16, tag="wk")
        wv_t = persist.tile([P, D * DKB // P], bf16, tag="wv")
        wo_t = persist.tile([P, DKB * D // P], bf16, tag="wo")
        tri_t = persist.tile([P, P], bf16, tag="tri")
        xq_t = persist.tile([P, nd, seq], bf16, tag="xq", name="xq")
        xk_t = persist.tile([P, nd, seq], bf16, tag="xk", name="xk")
        xv_t = persist.tile([P, nd, seq], bf16, tag="xv", name="xv")

        QT, KT, AT = [], [], []
        for m in range(2):
            QT.append(persist.tile([P, seq], bf16, tag=f"qt{m}", name=f"qt{m}"))
            KT.append(persist.tile([P, seq], bf16, tag=f"kt{m}", name=f"kt{m}"))
            AT.append(persist.tile([P, seq], bf16, tag=f"at{m}", name=f"at{m}"))
        vt = [
            persist.tile([P, HPC * (DK + 1)], bf16, tag=f"v{st}", name=f"v{st}")
            for st in range(nkt)
        ]

        # ---- input DMAs: enqueue cost is ~1.6us per descriptor, so spread
        # the enqueues across engines that are idle at kernel start ----
        # Single-engine enqueue: the ~1.6us/descriptor cost staggers the
        # transfers so earlier (more critical) DMAs get the HBM bandwidth
        # first. Strict consumption order.
        def load_x_chunk(xt, xd, n):
            nc.sync.dma_start(
                out=xt[:, :, n * QC : (n + 1) * QC],
                in_=xd[:, n * QC : (n + 1) * QC].rearrange(
                    "(j p) c -> p j c", p=P
                ),
            )

        load_x_chunk(xq_t, xq_d, 0)
        nc.sync.dma_start(out=wq_t[:], in_=wq_d[:, :])
        nc.sync.dma_start(out=wk_t[:], in_=wk_d[:, :])
        load_x_chunk(xk_t, xk_d, 0)
        load_x_chunk(xv_t, xv_d, 0)
        nc.sync.dma_start(out=wv_t[:], in_=wv_d[:, :])
        nc.sync.dma_start(out=tri_t[:], in_=tri_d[:, :])
        load_x_chunk(xq_t, xq_d, 1)
        load_x_chunk(xk_t, xk_d, 1)
        load_x_chunk(xv_t, xv_d, 1)
        load_x_chunk(xq_t, xq_d, 2)
        load_x_chunk(xk_t, xk_d, 2)
        load_x_chunk(xv_t, xv_d, 2)
        load_x_chunk(xq_t, xq_d, 3)
        load_x_chunk(xk_t, xk_d, 3)
        load_x_chunk(xv_t, xv_d, 3)
        nc.sync.dma_start(out=wo_t[:], in_=wo_d[:, :])

        fp_ps = ctx.enter_context(tc.tile_pool(name="fp_ps", bufs=2, space="PSUM"))
        st_ps_cm = tc.tile_pool(name="st_ps", bufs=2, space="PSUM")
        pv_ps_cm = tc.tile_pool(name="pv_ps", bufs=2, space="PSUM")
        st_ps = st_ps_cm.__enter__()
        pv_ps = pv_ps_cm.__enter__()
        atp = ctx.enter_context(tc.tile_pool(name="atp", bufs=6))
        smallp = ctx.enter_context(tc.tile_pool(name="smallp", bufs=4))
        obp = ctx.enter_context(tc.tile_pool(name="obp", bufs=2))

        # PE p-state warm-up: the DVFS governor needs ~18us of busy time
        # before the PE reaches 2.4GHz, and short gaps don't reset it.
        # Burn the first-DMA wait (~7us, which would otherwise be idle)
        # on dependency-free matmuls so the ramp clock starts early.
        dummy = persist.tile([P, QC], bf16, tag="dummy")
        nc.vector.memset(dummy[:], 0.5)
        wup = fp_ps.tile([P, QC], f32, tag="fp", name="warmup")
        for _ in range(16):
            nc.tensor.matmul(
                wup[:], lhsT=dummy[:, 0:P], rhs=dummy[:], start=True, stop=True
            )

        pending = []  # deferred (pv, hm, hp, qc, r) normalize emissions
        ob_cur = {}  # j2 -> ob tile being assembled

        def flush_pending():
            # bcast ln(sums) via a 1-cycle/col fp16 matmul, then exp(-x) on
            # scalar recovers 1/sums at fp32; emitted one head late so the
            # PE never waits on the chain
            while pending:
                pv, hm, hp, qc, lns = pending.pop(0)
                bcp = fp_ps.tile([DK, QC], f32, tag="fp", name=f"bcp{qc}_{hp}_{hm}")
                nc.tensor.matmul(
                    bcp[:],
                    lhsT=ones1[:],
                    rhs=lns[:],
                    start=True,
                    stop=True,
                )
                bc = smallp.tile([DK, QC], f32, tag="bc", name=f"bc{qc}_{hp}_{hm}")
                nc.scalar.activation(
                    out=bc[:], in_=bcp[:], func=exp_fn, scale=-1.0
                )
                nc.vector.tensor_mul(
                    AT[hm][hp : hp + DK, qc * QC : (qc + 1) * QC],
                    pv[0:DK, :],
                    bc[:],
                )

        def proj_qk_half(xt, wt, dest, n, m, dname):
            ps = fp_ps.tile([P, QC], f32, tag="fp", name=f"ps_{dname}{m}_{n}")
            for j in range(nd):
                nc.tensor.matmul(
                    ps[:],
                    lhsT=wt[:, j * DKB + m * P : j * DKB + (m + 1) * P],
                    rhs=xt[:, j, n * QC : (n + 1) * QC],
                    start=(j == 0),
                    stop=(j == nd - 1),
                )
            nc.vector.tensor_copy(
                out=dest[m][:, n * QC : (n + 1) * QC], in_=ps[:]
            )

        def proj_qk(xt, wt, dest, n, dname):
            for m in range(2):
                proj_qk_half(xt, wt, dest, n, m, dname)

        def proj_v_tile(st):
            ps = fp_ps.tile([P, DKB], f32, tag="fp", name=f"ps_v{st}")
            for j in range(nd):
                nc.tensor.matmul(
                    ps[:],
                    lhsT=xv_t[:, j, st * P : (st + 1) * P],
                    rhs=wv_t[:, j * DKB : (j + 1) * DKB],
                    start=(j == 0),
                    stop=(j == nd - 1),
                )
            v = vt[st]
            nc.vector.memset(v[:], 1.0)
            nc.vector.tensor_copy(
                out=v[:].rearrange("p (h w) -> p h w", w=DK + 1)[:, :, 0:DK],
                in_=ps[:].rearrange("p (h w) -> p h w", w=DK),
            )

        def outproj_quarter(qc, quarter, copy_on_scalar=False, pool=None):
            j2 = 2 * qc + quarter // 2
            g = quarter % 2
            st = 2 * j2 + g
            if g == 0:
                ob_cur[j2] = obp.tile([P, 2, D], f16, tag="ob", name=f"ob{j2}")
            ob = ob_cur[j2]
            for nch in range(2):
                ps = (pool or fp_ps).tile(
                    [P, QC], f32, tag="fp", name=f"ps_o{st}_{nch}"
                )
                for m in range(2):
                    nc.tensor.matmul(
                        ps[:],
                        lhsT=AT[m][:, st * P : (st + 1) * P],
                        rhs=wo_t[:, m * D + nch * QC : m * D + (nch + 1) * QC],
                        start=(m == 0),
                        stop=(m == 1),
                    )
                dst = ob[:, g, nch * QC : (nch + 1) * QC]
                if copy_on_scalar and nch == 0:
                    nc.scalar.activation(out=dst, in_=ps[:], func=copy_fn)
                else:
                    nc.vector.tensor_copy(out=dst, in_=ps[:])
            # one store per 256-row block: descriptor enqueue costs ~1.6us
            # of serial engine time, so fewer, larger stores win; the two
            # tail stores go on different engines to enqueue in parallel
            if g == 1:
                eng = nc.scalar if (copy_on_scalar and j2 % 2 == 1) else nc.sync
                eng.dma_start(
                    out=out_d[j2 * 2 * P : (j2 + 1) * 2 * P, :].rearrange(
                        "(g p) n -> p g n", p=P
                    ),
                    in_=ob[:],
                )
                del ob_cur[j2]

        def attn_head(qc, h, filler=None, interleave=(), mid=None):
            hm, hp = divmod(h, 2)
            hp *= DK
            kts = list(range(min(nkt, (qc + 1) * (QC // P))))
            pairs = [kts[i : i + 2] for i in range(0, len(kts), 2)]
            ats = []
            for pi, pair in enumerate(pairs):
                stt = st_ps.tile(
                    [P, 2 * QC], f32, tag="st", name=f"st{qc}_{h}_{pi}"
                )
                at = atp.tile([P, 2 * QC], bf16, tag="at", name=f"a{qc}_{h}_{pi}")
                diag_any = False
                for half, kt in enumerate(pair):
                    o = kt * P - qc * QC
                    diag = o >= 0
                    oo = max(o, 0)
                    diag_any |= diag
                    nc.tensor.matmul(
                        stt[:, half * QC + oo : (half + 1) * QC],
                        lhsT=KT[hm][hp : hp + DK, kt * P : (kt + 1) * P],
                        rhs=QT[hm][hp : hp + DK, qc * QC + oo : (qc + 1) * QC],
                        start=True,
                        stop=True,
                        skip_group_check=True,
                    )
                if pi < len(interleave):
                    interleave[pi]()
                if diag_any and pair[0] == 4 * qc:
                    # first diagonal pair: one exp over the whole pair; the
                    # 128-col hole holds bounded stale scores that are
                    # never read (PV starts past it)
                    nc.scalar.activation(
                        out=at[:], in_=stt[:], func=exp_fn, scale=0.125
                    )
                    for half, kt in enumerate(pair):
                        oo = max(kt * P - qc * QC, 0)
                        nc.vector.tensor_mul(
                            at[:, half * QC + oo : half * QC + oo + P],
                            at[:, half * QC + oo : half * QC + oo + P],
                            tri_t[:],
                        )
                elif diag_any:
                    # per-kt exp spans (trimmed); mask the 128-col diagonal
                    # band with the 0/1 lower-triangle tile after exp
                    for half, kt in enumerate(pair):
                        oo = max(kt * P - qc * QC, 0)
                        nc.scalar.activation(
                            out=at[:, half * QC + oo : (half + 1) * QC],
                            in_=stt[:, half * QC + oo : (half + 1) * QC],
                            func=exp_fn,
                            scale=0.125,
                        )
                        nc.vector.tensor_mul(
                            at[:, half * QC + oo : half * QC + oo + P],
                            at[:, half * QC + oo : half * QC + oo + P],
                            tri_t[:],
                        )
                else:
                    nc.scalar.activation(
                        out=at[:], in_=stt[:], func=exp_fn, scale=0.125
                    )
                ats.append((at, pair))
            for extra in interleave[len(pairs) :]:
                extra()
            # between scores and PV: always-ready projection filler work
            # (gives exp time to land without idling the PE), then the
            # previous head's normalize broadcast, then more ready work
            # (the previous q-chunk's outproj quarter) so the first PV
            # never races its exp
            if filler is not None:
                filler()
            flush_pending()
            if mid is not None:
                mid()
            pv = pv_ps.tile([DK + 1, QC], f32, tag="pv", name=f"pv{qc}_{h}")
            last_kt = kts[-1]
            for at, pair in ats:
                for half, kt in enumerate(pair):
                    oo = max(kt * P - qc * QC, 0)
                    nc.tensor.matmul(
                        pv[:, oo:QC],
                        lhsT=vt[kt][:, h * (DK + 1) : (h + 1) * (DK + 1)],
                        rhs=at[:, half * QC + oo : (half + 1) * QC],
                        start=(kt == 0),
                        stop=(kt == last_kt),
                        skip_group_check=True,
                    )
            lns = smallp.tile([1, QC], f16, tag="lns", name=f"lns{qc}_{h}")
            nc.scalar.activation(
                out=lns[:], in_=pv[DK : DK + 1, :], func=ln_fn
            )
            pending.append((pv, hm, hp, qc, lns))

        def attn(qc, fillers, pre=(), defer_last_quarter=False):
            for h in range(HPC):
                # h0's quarter reads AT rows written by this head's flush,
                # so it must trail the PVs; later heads' quarters are a
                # q-chunk old and slot in before the PVs as extra runway
                mid = None
                if qc > 0 and 0 < h and not (defer_last_quarter and h == HPC - 1):
                    mid = lambda h=h: outproj_quarter(qc - 1, h)
                attn_head(
                    qc,
                    h,
                    fillers[h] if h < len(fillers) else None,
                    interleave=pre if h == 0 else (),
                    mid=mid,
                )
                if qc > 0 and h == 0:
                    outproj_quarter(qc - 1, 0)
            if qc > 0 and defer_last_quarter:
                # always-ready PE work covering the last head's ln(sum)
                # latency before the final flush
                outproj_quarter(qc - 1, HPC - 1)

        def qk_fillers(n):
            return [
                lambda m=m, xt=xt, wt=wt, dst=dst, nm=nm: proj_qk_half(
                    xt, wt, dst, n, m, nm
                )
                for xt, wt, dst, nm in ((xq_t, wq_t, QT, "q"), (xk_t, wk_t, KT, "k"))
                for m in range(2)
            ]

        proj_qk(xq_t, wq_t, QT, 0, "q")
        proj_qk(xk_t, wk_t, KT, 0, "k")
        for n in range(nqc):
            pre = [
                (lambda st=st: proj_v_tile(st))
                for st in range(4 * n, 4 * n + 4)
            ]
            attn(
                n,
                qk_fillers(n + 1) if n + 1 < nqc else [],
                pre=pre,
                defer_last_quarter=(n + 1 == nqc),
            )
        flush_pending()
        # attention PSUM pools are done; reuse their banks for a deeper
        # tail pool so the final output projection streams without
        # rotation stalls
        pv_ps_cm.__exit__(None, None, None)
        st_ps_cm.__exit__(None, None, None)
        with tc.tile_pool(name="tailp", bufs=4, space="PSUM") as tailp:
            # dependency-free burst hides the last head's normalize (DVE
            # mul) latency before the tail projection reads AT
            wup2 = tailp.tile([P, QC], f32, tag="fp", name="tailwarm")
            for _ in range(8):
                nc.tensor.matmul(
                    wup2[:],
                    lhsT=dummy[:, 0:P],
                    rhs=dummy[:],
                    start=True,
                    stop=True,
                )
            for q in range(4):
                outproj_quarter(nqc - 1, q, copy_on_scalar=True, pool=tailp)

    return nc


def _build(mask_mode, seq=S):
    """v1 builder kept for the 'none'/'full' mask fallbacks."""
    import concourse.bass as bass
    import concourse.tile as tile
    from concourse import mybir
    from contextlib import ExitStack

    f32 = mybir.dt.float32
    bf16 = mybir.dt.bfloat16
    nqc = seq // QC
    nkt = seq // P
    nd = D // P  # 8 d-chunks

    nc = bass.Bass(num_swdge_queues=4)
    xq_d = nc.dram_tensor("xq_t", [D, seq], bf16, kind="ExternalInput")
    xk_d = nc.dram_tensor("xk_t", [D, seq], bf16, kind="ExternalInput")
    xv_d = nc.dram_tensor("xv_t", [D, seq], bf16, kind="ExternalInput")
    wq_d = nc.dram_tensor("wq_p", [P, D * DKB // P], bf16, kind="ExternalInput")
    wk_d = nc.dram_tensor("wk_p", [P, D * DKB // P], bf16, kind="ExternalInput")
    wv_d = nc.dram_tensor("wv_p", [P, D * DKB // P], bf16, kind="ExternalInput")
    wo_d = nc.dram_tensor("wo_p", [P, DKB * D // P], bf16, kind="ExternalInput")
    if mask_mode == "causal":
        stair_d = nc.dram_tensor("stair", [P, QC + 384], bf16, kind="ExternalInput")
    if mask_mode == "full":
        maskt_d = nc.dram_tensor("mask_t", [seq, seq], bf16, kind="ExternalInput")
    out_d = nc.dram_tensor("out", [seq, D], f32, kind="ExternalOutput")

    with ExitStack() as ctx:
        tc = ctx.enter_context(tile.TileContext(nc))
        persist = ctx.enter_context(tc.tile_pool(name="persist", bufs=1))

        ones64 = persist.tile([1, DK], f32, tag="ones64")
        nc.vector.memset(ones64[:], 1.0)
        wq_t = persist.tile([P, D * DKB // P], bf16, tag="wq")
        wk_t = persist.tile([P, D * DKB // P], bf16, tag="wk")
        wv_t = persist.tile([P, D * DKB // P], bf16, tag="wv")
        wo_t = persist.tile([P, DKB * D // P], bf16, tag="wo")
        nc.gpsimd.dma_start(out=wq_t[:], in_=wq_d[:, :])
        nc.gpsimd.dma_start(out=wk_t[:], in_=wk_d[:, :])
        nc.gpsimd.dma_start(out=wv_t[:], in_=wv_d[:, :])
        nc.gpsimd.dma_start(out=wo_t[:], in_=wo_d[:, :])
        if mask_mode == "causal":
            stair_t = persist.tile([P, QC + 384], bf16, tag="stair")
            nc.gpsimd.dma_start(out=stair_t[:], in_=stair_d[:, :])

        QT, KT, vt = [], [], []
        for m in range(2):
            QT.append(persist.tile([P, seq], bf16, tag=f"qt{m}", name=f"qt{m}"))
            KT.append(persist.tile([P, seq], bf16, tag=f"kt{m}", name=f"kt{m}"))
        AT = []
        for m in range(2):
            AT.append(persist.tile([P, seq], bf16, tag=f"at{m}", name=f"at{m}"))

        # ---- phase 1: projections (own PSUM + x pools, released after) ----
        with tc.tile_pool(name="xpool", bufs=1) as xpool, tc.tile_pool(
            name="projp", bufs=2, space="PSUM"
        ) as projp:

            def load_xt(xdram, name):
                t = xpool.tile([P, nd, seq], bf16, tag=name, name=name)
                h = nd // 2
                nc.sync.dma_start(
                    out=t[:, 0:h, :],
                    in_=xdram[: h * P, :].rearrange("(j p) s -> p j s", p=P),
                )
                nc.sync.dma_start(
                    out=t[:, h:nd, :],
                    in_=xdram[h * P :, :].rearrange("(j p) s -> p j s", p=P),
                )
                return t

            xq_t = load_xt(xq_d, "xq")
            xk_t = load_xt(xk_d, "xk")
            xv_t = load_xt(xv_d, "xv")

            def project_T(xt, wtile, res, name):
                ngroups = [
                    list(range(i, min(i + 2, nqc))) for i in range(0, nqc, 2)
                ]
                for m in range(2):
                    for gi, grp in enumerate(ngroups):
                        ps = projp.tile(
                            [P, len(grp) * QC],
                            f32,
                            tag="pj",
                            name=f"ps_{name}{m}_{gi}",
                        )
                        for half, n in enumerate(grp):
                            for j in range(nd):
                                nc.tensor.matmul(
                                    ps[:, half * QC : (half + 1) * QC],
                                    lhsT=wtile[
                                        :, j * DKB + m * P : j * DKB + (m + 1) * P
                                    ],
                                    rhs=xt[:, j, n * QC : (n + 1) * QC],
                                    start=(j == 0),
                                    stop=(j == nd - 1),
                                )
                        nc.vector.tensor_copy(
                            out=res[m][:, grp[0] * QC : (grp[-1] + 1) * QC],
                            in_=ps[:],
                        )

            project_T(xq_t, wq_t, QT, "qt")
            project_T(xk_t, wk_t, KT, "kt")

            # V natural layout [s, dv] + ones column per head
            for st in range(nkt):
                ps = projp.tile([P, DKB], f32, tag="pj", name=f"ps_v{st}")
                for j in range(nd):
                    nc.tensor.matmul(
                        ps[:],
                        lhsT=xv_t[:, j, st * P : (st + 1) * P],
                        rhs=wv_t[:, j * DKB : (j + 1) * DKB],
                        start=(j == 0),
                        stop=(j == nd - 1),
                    )
                v = persist.tile(
                    [P, HPC * (DK + 1)], bf16, tag=f"v{st}", name=f"v{st}"
                )
                nc.vector.memset(v[:], 1.0)
                nc.vector.tensor_copy(
                    out=v[:].rearrange("p (h w) -> p h w", w=DK + 1)[:, :, 0:DK],
                    in_=ps[:].rearrange("p (h w) -> p h w", w=DK),
                )
                vt.append(v)

        # ---- phase 2: attention (+ per-qc output projection) ----
        st_ps = ctx.enter_context(tc.tile_pool(name="st_ps", bufs=4, space="PSUM"))
        pv_ps = ctx.enter_context(tc.tile_pool(name="pv_ps", bufs=2, space="PSUM"))
        fp_ps = ctx.enter_context(tc.tile_pool(name="fp_ps", bufs=2, space="PSUM"))
        sc_pool = ctx.enter_context(tc.tile_pool(name="sc_pool", bufs=8))
        attn_pool = ctx.enter_context(tc.tile_pool(name="attn_pool", bufs=8))
        small = ctx.enter_context(tc.tile_pool(name="small", bufs=2))
        outp = ctx.enter_context(tc.tile_pool(name="outp", bufs=2))
        maskp = None
        if mask_mode == "full":
            maskp = ctx.enter_context(tc.tile_pool(name="maskp", bufs=2))

        exp_fn = mybir.ActivationFunctionType.Exp
        ln_fn = mybir.ActivationFunctionType.Ln
        for qc in range(nqc):
            mt = None
            if mask_mode == "full":
                mt = maskp.tile([P, nkt, QC], bf16, tag="mask", name=f"mt{qc}")
                nc.gpsimd.dma_start(
                    out=mt[:],
                    in_=maskt_d[:, qc * QC : (qc + 1) * QC].rearrange(
                        "(kt p) c -> p kt c", p=P
                    ),
                )
            for h in range(HPC):
                hm, hp = divmod(h, 2)
                hp *= DK
                if mask_mode == "causal":
                    kts = list(range(min(nkt, (qc + 1) * (QC // P))))
                else:
                    kts = list(range(nkt))
                pairs = [kts[i : i + 2] for i in range(0, len(kts), 2)]

                def trim(kt):
                    if mask_mode != "causal":
                        return 0, False
                    o = kt * P - qc * QC
                    return (o, True) if o >= 0 else (0, False)

                pv = pv_ps.tile([DK + 1, QC], f32, tag="pv", name=f"pv{qc}_{h}")

                def emit_pv(at, pair, is_last):
                    for half, kt in enumerate(pair):
                        o, _ = trim(kt)
                        nc.tensor.matmul(
                            pv[:, o:QC],
                            lhsT=vt[kt][:, h * (DK + 1) : (h + 1) * (DK + 1)],
                            rhs=at[:, half * QC + o : (half + 1) * QC],
                            start=(kt == 0),
                            stop=(is_last and half == len(pair) - 1),
                            skip_group_check=True,
                        )

                ats = []
                for pi, pair in enumerate(pairs):
                    sc = sc_pool.tile(
                        [P, 2 * QC], f32, tag="sc", name=f"sc{qc}_{h}_{pi}"
                    )
                    for half, kt in enumerate(pair):
                        o, diag = trim(kt)
                        stt = st_ps.tile(
                            [P, QC], f32, tag="st", name=f"st{qc}_{h}_{kt}"
                        )
                        nc.tensor.matmul(
                            stt[:, o:QC],
                            lhsT=KT[hm][hp : hp + DK, kt * P : (kt + 1) * P],
                            rhs=QT[hm][
                                hp : hp + DK, qc * QC + o : (qc + 1) * QC
                            ],
                            start=True,
                            stop=True,
                        )
                        dst = sc[:, half * QC + o : (half + 1) * QC]
                        if diag:
                            nc.vector.tensor_add(
                                out=dst,
                                in0=stt[:, o:QC],
                                in1=stair_t[:, 384 : 384 + QC - o],
                            )
                        elif mask_mode == "full":
                            nc.vector.tensor_add(
                                out=dst, in0=stt[:, o:QC], in1=mt[:, kt, :]
                            )
                        else:
                            nc.vector.tensor_copy(out=dst, in_=stt[:, o:QC])
                    at = attn_pool.tile(
                        [P, 2 * QC], bf16, tag="attn", name=f"a{qc}_{h}_{pi}"
                    )
                    o0, _ = trim(pair[0])
                    o1 = trim(pair[1])[0] if len(pair) > 1 else 0
                    if o1 == 0:
                        spans = [(o0, len(pair) * QC)]
                    else:
                        spans = [(o0, QC), (QC + o1, 2 * QC)]
                    for lo, hi in spans:
                        nc.scalar.activation(
                            out=at[:, lo:hi],
                            in_=sc[:, lo:hi],
                            func=exp_fn,
                            scale=0.125,
                        )
                    ats.append((at, pair))
                for at, pair in ats:
                    emit_pv(at, pair, pair is pairs[-1])
                # normalize: 1/sums = exp(-ln(sums)), broadcast via PE
                lns = small.tile([1, QC], f32, tag="lns", name=f"ln{qc}_{h}")
                nc.scalar.activation(
                    out=lns[:], in_=pv[DK : DK + 1, :], func=ln_fn
                )
                bcp = fp_ps.tile([DK, QC], f32, tag="fp", name=f"bcp{qc}_{h}")
                nc.tensor.matmul(
                    bcp[:], lhsT=ones64[:], rhs=lns[:], start=True, stop=True
                )
                bc = small.tile([DK, QC], f32, tag="bcast", name=f"bc{qc}_{h}")
                nc.scalar.activation(
                    out=bc[:], in_=bcp[:], func=exp_fn, scale=-1.0
                )
                nc.vector.tensor_mul(
                    AT[hm][hp : hp + DK, qc * QC : (qc + 1) * QC],
                    pv[0:DK, :],
                    bc[:],
                )

            # ---- output projection for this qc's s-tiles (keeps PE on
            # full-array matmuls between attention chunks; removes tail) ----
            for j2 in range(qc * (QC // (2 * P)), (qc + 1) * (QC // (2 * P))):
                ob = outp.tile([P, 2, D], f32, tag="ob", name=f"ob{j2}")
                for g in range(2):
                    st = 2 * j2 + g
                    for nch in range(D // QC):
                        ps = fp_ps.tile(
                            [P, QC], f32, tag="fp", name=f"ps_o{st}_{nch}"
                        )
                        for m in range(2):
                            nc.tensor.matmul(
                                ps[:],
                                lhsT=AT[m][:, st * P : (st + 1) * P],
                                rhs=wo_t[
                                    :, m * D + nch * QC : m * D + (nch + 1) * QC
                                ],
                                start=(m == 0),
                                stop=(m == 1),
                            )
                        nc.vector.tensor_copy(
                            out=ob[:, g, nch * QC : (nch + 1) * QC], in_=ps[:]
                        )
                nc.sync.dma_start(
                    out=out_d[j2 * 2 * P : (j2 + 1) * 2 * P, :].rearrange(
                        "(g p) n -> p g n", p=P
                    ),
                    in_=ob[:],
                )

    return nc


def _split_multi_waits(nc):
    """This toolchain's walrus accepts at most one sync-wait per
    instruction. Hoist extra waits onto preceding same-engine NoOps —
    engine streams execute in order, so a NoOp that blocks on a
    semaphore gates everything after it (including HWDGE descriptor
    enqueues, which happen when the issuing engine's sequencer reaches
    the DMA instruction)."""
    import bass_rust

    ctr = 0
    for f in nc.m.functions:
        for bb in f.blocks:
            insts = bb.instructions
            new = []
            changed = False
            for inst in insts:
                si = inst.sync_info
                if si is not None and len(si.on_wait) > 1:
                    waits = list(si.on_wait)
                    for w in waits[:-1]:
                        ctr += 1
                        nop = bass_rust.InstNoOp(
                            name=f"wsplit_{ctr}", ins=[], outs=[]
                        )
                        nop.engine = inst.engine
                        nop.sync_info = bass_rust.SyncInfo(
                            on_wait=[w], on_update=[]
                        )
                        new.append(nop)
                    inst.sync_info = bass_rust.SyncInfo(
                        on_wait=[waits[-1]], on_update=list(si.on_update)
                    )
                    changed = True
                new.append(inst)
            if changed:
                try:
                    bb.instructions = new
                except AttributeError:
                    insts.clear()
                    insts.extend(new)
    return nc


def _get_nc(mask_mode, seq=S, split_waits=True):
    key = (mask_mode, seq, split_waits)
    if key not in _nc_cache:
        if mask_mode == "causal":
            nc = _build_causal(seq)
        else:
            nc = _build(mask_mode, seq)
        if split_waits:
            _split_multi_waits(nc)
        _nc_cache[key] = nc
    return _nc_cache[key]


def _pack_w(w_slice_T, ncols):
    # [D_in, ncols] -> [128, D_in/128 * ncols]: col block j holds rows j*128..
    d_in = w_slice_T.shape[0]
    return (
        w_slice_T.reshape(d_in // P, P, ncols).transpose(1, 0, 2).reshape(P, -1)
    )


def _stair_np():
    cc = np.arange(QC + 384)[None, :]
    r = np.arange(P)[:, None]
    return np.where(cc < r + 384, np.float32(NEG), np.float32(0.0)).astype(BF16)


def _tri_np():
    # 0 where col < row (strictly below the shifted diagonal = masked)
    c = np.arange(P)[None, :]
    r = np.arange(P)[:, None]
    return np.where(c < r, np.float32(0.0), np.float32(1.0)).astype(BF16)


def _detect_mask_mode(mask):
    if not mask.any():
        return "none"
    causal = np.triu(np.ones((mask.shape[1], mask.shape[2]), bool), k=1)
    if all(np.array_equal(mask[b], causal) for b in range(mask.shape[0])):
        return "causal"
    return "full"


def _make_in_maps(query, key, value, mask, w_q, w_k, w_v, w_o, mask_mode, seq=S):
    per_batch = []
    for b in range(B):
        d = {
            "xq_t": np.ascontiguousarray(query[b].T).astype(BF16),
            "xk_t": np.ascontiguousarray(key[b].T).astype(BF16),
            "xv_t": np.ascontiguousarray(value[b].T).astype(BF16),
        }
        if mask_mode == "full":
            d["mask_t"] = np.where(
                mask[b].T, np.float32(NEG), np.float32(0.0)
            ).astype(BF16)
        per_batch.append(d)
    per_hg = []
    for hg in range(HGROUPS):
        rows = slice(hg * DKB, (hg + 1) * DKB)
        per_hg.append(
            {
                "wq_p": _pack_w(w_q[rows, :].T.astype(BF16), DKB),
                "wk_p": _pack_w(w_k[rows, :].T.astype(BF16), DKB),
                "wv_p": _pack_w(w_v[rows, :].T.astype(BF16), DKB),
                "wo_p": _pack_w(w_o[:, rows].T.astype(BF16), D),
            }
        )
    if mask_mode == "causal":
        extra = {"tri01": _tri_np()}
    elif mask_mode == "none":
        extra = {}
    else:
        extra = {}
    in_maps = []
    for c in range(NCORE):
        b, hg = divmod(c, HGROUPS)
        im = dict(per_batch[b])
        im.update(per_hg[hg])
        im.update(extra)
        in_maps.append(im)
    return in_maps


def _run(inputs, trace=False):
    from concourse.bass_utils import run_bass_kernel_spmd

    query = np.asarray(inputs["query"], np.float32)
    key = np.asarray(inputs["key"], np.float32)
    value = np.asarray(inputs["value"], np.float32)
    mask = np.asarray(inputs["mask"], bool)
    w_q = np.asarray(inputs["w_q"], np.float32)
    w_k = np.asarray(inputs["w_k"], np.float32)
    w_v = np.asarray(inputs["w_v"], np.float32)
    w_o = np.asarray(inputs["w_o"], np.float32)
    b_o = np.asarray(inputs["b_o"], np.float32)
    assert query.shape == (B, S, D), query.shape

    mask_mode = _detect_mask_mode(mask)
    nc = _get_nc(mask_mode)
    in_maps = _make_in_maps(query, key, value, mask, w_q, w_k, w_v, w_o, mask_mode)
    res = run_bass_kernel_spmd(nc, in_maps, list(range(NCORE)), trace=trace)
    outs = [np.asarray(r["out"], np.float32) for r in res.results]
    full = np.empty((B, S, D), np.float32)
    for b in range(B):
        full[b] = outs[HGROUPS * b]
        for i in range(1, HGROUPS):
            full[b] += outs[HGROUPS * b + i]
    full += b_o[None, None, :]
    return full, res


def kernel(**inputs):
    out, _ = _run(inputs, trace=False)
    return out


if __name__ == "__main__":
    import tempfile
    from concourse.bass_utils import compile_bass_kernel

    mode = sys.argv[1] if len(sys.argv) > 1 else "causal"
    nc = _get_nc(mode)
    from collections import Counter

    c = Counter()
    for name, inst in nc.inst_map.items():
        if "DMACopy" in type(inst).__name__:
            c[str(inst).count("wait:")] += 1
    print("DMA wait dist:", dict(c))
    td = tempfile.mkdtemp()
    p = compile_bass_kernel(nc, td)
    print("COMPILED OK:", p)


# revision 60
# speedup vs baseline: 1.1846x; 1.0128x over previous
"""Multi-head attention (B=2, S=2048, D=1024, H=16, causal mask) on 8 TRN2 cores.

Sharding: core c handles batch b = c // 4 and head-group hg = c % 4
(4 heads = 256 feature dims each). Each core computes its heads' QKV
projections, causal attention, and a partial output projection
(attn_out @ w_o[:, hg].T); the host sums the 4 partials per batch and
adds b_o.

v2 schedule: the PE p-state ramp (0.65->1.2->2.4 GHz, max only after
~3us of gap-free execution) dominates performance, so the kernel is
organized as one continuous PE stream with no cross-engine round
trips on the critical path:

  - projections are chunked by seq quarter n and software-pipelined
    with attention: proj(n) -> attn(qc=n-1 tail norms + qc=n) so the
    PE always has dense 128-contraction work between attention deps
  - exp reads score PSUM directly (no DVE staging copy); causal
    masking is a single [128,128] {0,1} lower-triangle multiply on
    the 128-col diagonal band of each diagonal block, after exp
  - softmax normalize: DVE fast-reciprocal of the PV ones-column row,
    broadcast via a 1-row f32r matmul (1 cycle/col) emitted one head
    late so the PE never waits on it, then one DVE multiply
  - output projection for q-chunk qc-1 is interleaved between the
    heads of q-chunk qc (extra always-ready PE work + spread-out
    output DMA); out is stored fp16
  - all HBM traffic on HWDGE queues, inputs chunked and enqueued from
    one engine in consumption order (the ~1.6us/descriptor enqueue
    cost staggers transfers, prioritizing the critical path)
  - the PE DVFS governor needs ~5.5us of full-array busy to reach
    2.4GHz: 16 dependency-free full-array warmup matmuls burn the
    initial DMA wait to pre-ramp it; a second burst hides the last
    head's normalize latency before the tail projection
  - attention PSUM pools close before the tail so the final output
    projection gets a 4-deep pool (no rotation stalls)

DMA discipline: this toolchain rejects DMA instructions with >1 sync
wait, and the Tile layer adds a ring-credit wait from the 3rd use of
each of the 8 HWDGE queues; _split_multi_waits hoists extra waits
onto same-engine NoOps. The general mask fallback keeps the v1 code.
"""

import sys

if "/opt/trn_rl_repo" not in sys.path:
    sys.path.insert(0, "/opt/trn_rl_repo")

import numpy as np
import ml_dtypes

BF16 = ml_dtypes.bfloat16
F16 = np.float16

B, S, D, H = 2, 2048, 1024, 16
NCORE = 8
HGROUPS = 4  # head-groups == cores per batch
HPC = H // HGROUPS  # heads per core = 4
DK = D // H  # head dim = 64
DKB = HPC * DK  # feature dims per core = 256
P = 128
QC = 512  # q chunk (one PSUM bank of fp32)
NEG = -1e9

_nc_cache = {}


def _build_causal(seq=S):
    """Fast causal-mask kernel (see module docstring)."""
    import concourse.bass as bass
    import concourse.tile as tile
    from concourse import mybir
    from contextlib import ExitStack

    f32 = mybir.dt.float32
    f16 = mybir.dt.float16
    bf16 = mybir.dt.bfloat16
    exp_fn = mybir.ActivationFunctionType.Exp
    ln_fn = mybir.ActivationFunctionType.Ln
    copy_fn = mybir.ActivationFunctionType.Copy
    nqc = seq // QC  # 4
    nkt = seq // P  # 16
    nd = D // P  # 8

    nc = bass.Bass(num_swdge_queues=1)
    xq_d = nc.dram_tensor("xq_t", [D, seq], bf16, kind="ExternalInput")
    xk_d = nc.dram_tensor("xk_t", [D, seq], bf16, kind="ExternalInput")
    xv_d = nc.dram_tensor("xv_t", [D, seq], bf16, kind="ExternalInput")
    wq_d = nc.dram_tensor("wq_p", [P, D * DKB // P], bf16, kind="ExternalInput")
    wk_d = nc.dram_tensor("wk_p", [P, D * DKB // P], bf16, kind="ExternalInput")
    wv_d = nc.dram_tensor("wv_p", [P, D * DKB // P], bf16, kind="ExternalInput")
    wo_d = nc.dram_tensor("wo_p", [P, DKB * D // P], bf16, kind="ExternalInput")
    tri_d = nc.dram_tensor("tri01", [P, P], bf16, kind="ExternalInput")
    out_d = nc.dram_tensor("out", [seq, D], f16, kind="ExternalOutput")

    with ExitStack() as ctx:
        tc = ctx.enter_context(tile.TileContext(nc))
        persist = ctx.enter_context(tc.tile_pool(name="persist", bufs=1))

        ones1 = persist.tile([1, DK], f16, tag="ones1")
        nc.vector.memset(ones1[:], 1.0)
        wq_t = persist.tile([P, D * DKB // P], bf16, tag="wq")
        wk_t = persist.tile([P, D * DKB // P], bf16, tag="wk")
        wv_t = persist.tile([P, D * DKB // P], bf16, tag="wv")
        wo_t = persist.tile([P, DKB * D // P], bf16, tag="wo")
        tri_t = persist.tile([P, P], bf16, tag="tri")
        xq_t = persist.tile([P, nd, seq], bf16, tag="xq", name="xq")
        xk_t = persist.tile([P, nd, seq], bf16, tag="xk", name="xk")
        xv_t = persist.tile([P, nd, seq], bf16, tag="xv", name="xv")

        QT, KT, AT = [], [], []
        for m in range(2):
            QT.append(persist.tile([P, seq], bf16, tag=f"qt{m}", name=f"qt{m}"))
            KT.append(persist.tile([P, seq], bf16, tag=f"kt{m}", name=f"kt{m}"))
            AT.append(persist.tile([P, seq], bf16, tag=f"at{m}", name=f"at{m}"))
        vt = [
            persist.tile([P, HPC * (DK + 1)], bf16, tag=f"v{st}", name=f"v{st}")
            for st in range(nkt)
        ]

        # ---- input DMAs: enqueue cost is ~1.6us per descriptor, so spread
        # the enqueues across engines that are idle at kernel start ----
        # Single-engine enqueue: the ~1.6us/descriptor cost staggers the
        # transfers so earlier (more critical) DMAs get the HBM bandwidth
        # first. Strict consumption order.
        def load_x_chunk(xt, xd, n):
            nc.sync.dma_start(
                out=xt[:, :, n * QC : (n + 1) * QC],
                in_=xd[:, n * QC : (n + 1) * QC].rearrange(
                    "(j p) c -> p j c", p=P
                ),
            )

        load_x_chunk(xq_t, xq_d, 0)
        nc.sync.dma_start(out=wq_t[:], in_=wq_d[:, :])
        nc.sync.dma_start(out=wk_t[:], in_=wk_d[:, :])
        load_x_chunk(xk_t, xk_d, 0)
        load_x_chunk(xv_t, xv_d, 0)
        nc.sync.dma_start(out=wv_t[:], in_=wv_d[:, :])
        nc.sync.dma_start(out=tri_t[:], in_=tri_d[:, :])
        load_x_chunk(xq_t, xq_d, 1)
        load_x_chunk(xk_t, xk_d, 1)
        load_x_chunk(xv_t, xv_d, 1)
        load_x_chunk(xq_t, xq_d, 2)
        load_x_chunk(xk_t, xk_d, 2)
        load_x_chunk(xv_t, xv_d, 2)
        load_x_chunk(xq_t, xq_d, 3)
        load_x_chunk(xk_t, xk_d, 3)
        load_x_chunk(xv_t, xv_d, 3)
        nc.sync.dma_start(out=wo_t[:], in_=wo_d[:, :])

        fp_ps = ctx.enter_context(tc.tile_pool(name="fp_ps", bufs=2, space="PSUM"))
        st_ps_cm = tc.tile_pool(name="st_ps", bufs=2, space="PSUM")
        pv_ps_cm = tc.tile_pool(name="pv_ps", bufs=2, space="PSUM")
        st_ps = st_ps_cm.__enter__()
        pv_ps = pv_ps_cm.__enter__()
        atp = ctx.enter_context(tc.tile_pool(name="atp", bufs=6))
        smallp = ctx.enter_context(tc.tile_pool(name="smallp", bufs=4))
        obp = ctx.enter_context(tc.tile_pool(name="obp", bufs=2))

        # PE p-state warm-up: the DVFS governor needs ~18us of busy time
        # before the PE reaches 2.4GHz, and short gaps don't reset it.
        # Burn the first-DMA wait (~7us, which would otherwise be idle)
        # on dependency-free matmuls so the ramp clock starts early.
        dummy = persist.tile([P, QC], bf16, tag="dummy")
        nc.vector.memset(dummy[:], 0.5)
        wup = fp_ps.tile([P, QC], f32, tag="fp", name="warmup")
        for _ in range(16):
            nc.tensor.matmul(
                wup[:], lhsT=dummy[:, 0:P], rhs=dummy[:], start=True, stop=True
            )

        pending = []  # deferred (pv, hm, hp, qc, r) normalize emissions
        ob_cur = {}  # j2 -> ob tile being assembled

        def flush_pending():
            # bcast ln(sums) via a 1-cycle/col fp16 matmul, then exp(-x) on
            # scalar recovers 1/sums at fp32; emitted one head late so the
            # PE never waits on the chain
            while pending:
                pv, hm, hp, qc, lns = pending.pop(0)
                bcp = fp_ps.tile([DK, QC], f32, tag="fp", name=f"bcp{qc}_{hp}_{hm}")
                nc.tensor.matmul(
                    bcp[:],
                    lhsT=ones1[:],
                    rhs=lns[:],
                    start=True,
                    stop=True,
                )
                bc = smallp.tile([DK, QC], f32, tag="bc", name=f"bc{qc}_{hp}_{hm}")
                nc.scalar.activation(
                    out=bc[:], in_=bcp[:], func=exp_fn, scale=-1.0
                )
                nc.vector.tensor_mul(
                    AT[hm][hp : hp + DK, qc * QC : (qc + 1) * QC],
                    pv[0:DK, :],
                    bc[:],
                )

        def proj_qk_half(xt, wt, dest, n, m, dname):
            ps = fp_ps.tile([P, QC], f32, tag="fp", name=f"ps_{dname}{m}_{n}")
            for j in range(nd):
                nc.tensor.matmul(
                    ps[:],
                    lhsT=wt[:, j * DKB + m * P : j * DKB + (m + 1) * P],
                    rhs=xt[:, j, n * QC : (n + 1) * QC],
                    start=(j == 0),
                    stop=(j == nd - 1),
                )
            nc.vector.tensor_copy(
                out=dest[m][:, n * QC : (n + 1) * QC], in_=ps[:]
            )

        def proj_qk(xt, wt, dest, n, dname):
            for m in range(2):
                proj_qk_half(xt, wt, dest, n, m, dname)

        def proj_v_tile(st):
            ps = fp_ps.tile([P, DKB], f32, tag="fp", name=f"ps_v{st}")
            for j in range(nd):
                nc.tensor.matmul(
                    ps[:],
                    lhsT=xv_t[:, j, st * P : (st + 1) * P],
                    rhs=wv_t[:, j * DKB : (j + 1) * DKB],
                    start=(j == 0),
                    stop=(j == nd - 1),
                )
            v = vt[st]
            nc.vector.memset(v[:], 1.0)
            nc.vector.tensor_copy(
                out=v[:].rearrange("p (h w) -> p h w", w=DK + 1)[:, :, 0:DK],
                in_=ps[:].rearrange("p (h w) -> p h w", w=DK),
            )

        def outproj_quarter(qc, quarter, copy_on_scalar=False, pool=None):
            j2 = 2 * qc + quarter // 2
            g = quarter % 2
            st = 2 * j2 + g
            if g == 0:
                ob_cur[j2] = obp.tile([P, 2, D], f16, tag="ob", name=f"ob{j2}")
            ob = ob_cur[j2]
            for nch in range(2):
                ps = (pool or fp_ps).tile(
                    [P, QC], f32, tag="fp", name=f"ps_o{st}_{nch}"
                )
                for m in range(2):
                    nc.tensor.matmul(
                        ps[:],
                        lhsT=AT[m][:, st * P : (st + 1) * P],
                        rhs=wo_t[:, m * D + nch * QC : m * D + (nch + 1) * QC],
                        start=(m == 0),
                        stop=(m == 1),
                    )
                dst = ob[:, g, nch * QC : (nch + 1) * QC]
                if copy_on_scalar and nch == 0:
                    nc.scalar.activation(out=dst, in_=ps[:], func=copy_fn)
                else:
                    nc.vector.tensor_copy(out=dst, in_=ps[:])
            # one store per 256-row block: descriptor enqueue costs ~1.6us
            # of serial engine time, so fewer, larger stores win; the two
            # tail stores go on different engines to enqueue in parallel
            if g == 1:
                eng = nc.scalar if (copy_on_scalar and j2 % 2 == 1) else nc.sync
                eng.dma_start(
                    out=out_d[j2 * 2 * P : (j2 + 1) * 2 * P, :].rearrange(
                        "(g p) n -> p g n", p=P
                    ),
                    in_=ob[:],
                )
                del ob_cur[j2]

        def attn_head(qc, h, filler=None, interleave=(), mid=None):
            hm, hp = divmod(h, 2)
            hp *= DK
            kts = list(range(min(nkt, (qc + 1) * (QC // P))))
            pairs = [kts[i : i + 2] for i in range(0, len(kts), 2)]
            ats = []
            for pi, pair in enumerate(pairs):
                stt = st_ps.tile(
                    [P, 2 * QC], f32, tag="st", name=f"st{qc}_{h}_{pi}"
                )
                at = atp.tile([P, 2 * QC], bf16, tag="at", name=f"a{qc}_{h}_{pi}")
                diag_any = False
                for half, kt in enumerate(pair):
                    o = kt * P - qc * QC
                    diag = o >= 0
                    oo = max(o, 0)
                    diag_any |= diag
                    nc.tensor.matmul(
                        stt[:, half * QC + oo : (half + 1) * QC],
                        lhsT=KT[hm][hp : hp + DK, kt * P : (kt + 1) * P],
                        rhs=QT[hm][hp : hp + DK, qc * QC + oo : (qc + 1) * QC],
                        start=True,
                        stop=True,
                        skip_group_check=True,
                    )
                if pi < len(interleave):
                    interleave[pi]()
                if diag_any and pair[0] == 4 * qc:
                    # first diagonal pair: one exp over the whole pair; the
                    # 128-col hole holds bounded stale scores that are
                    # never read (PV starts past it)
                    nc.scalar.activation(
                        out=at[:], in_=stt[:], func=exp_fn, scale=0.125
                    )
                    for half, kt in enumerate(pair):
                        oo = max(kt * P - qc * QC, 0)
                        nc.vector.tensor_mul(
                            at[:, half * QC + oo : half * QC + oo + P],
                            at[:, half * QC + oo : half * QC + oo + P],
                            tri_t[:],
                        )
                elif diag_any:
                    # per-kt exp spans (trimmed); mask the 128-col diagonal
                    # band with the 0/1 lower-triangle tile after exp
                    for half, kt in enumerate(pair):
                        oo = max(kt * P - qc * QC, 0)
                        nc.scalar.activation(
                            out=at[:, half * QC + oo : (half + 1) * QC],
                            in_=stt[:, half * QC + oo : (half + 1) * QC],
                            func=exp_fn,
                            scale=0.125,
                        )
                        nc.vector.tensor_mul(
                            at[:, half * QC + oo : half * QC + oo + P],
                            at[:, half * QC + oo : half * QC + oo + P],
                            tri_t[:],
                        )
                else:
                    nc.scalar.activation(
                        out=at[:], in_=stt[:], func=exp_fn, scale=0.125
                    )
                ats.append((at, pair))
            for extra in interleave[len(pairs) :]:
                extra()
            # between scores and PV: always-ready projection filler work
            # (gives exp time to land without idling the PE), then the
            # previous q-chunk's outproj quarter, then the normalize
            # broadcast. Quarter-before-flush orders the fp-PSUM ring so
            # no outproj matmul waits on the scalar bc-exp freeing bcp.
            if filler is not None:
                filler()
            if mid is not None:
                mid()
            flush_pending()
            pv = pv_ps.tile([DK + 1, QC], f32, tag="pv", name=f"pv{qc}_{h}")
            last_kt = kts[-1]
            for at, pair in ats:
                for half, kt in enumerate(pair):
                    oo = max(kt * P - qc * QC, 0)
                    nc.tensor.matmul(
                        pv[:, oo:QC],
                        lhsT=vt[kt][:, h * (DK + 1) : (h + 1) * (DK + 1)],
                        rhs=at[:, half * QC + oo : (half + 1) * QC],
                        start=(kt == 0),
                        stop=(kt == last_kt),
                        skip_group_check=True,
                    )
            lns = smallp.tile([1, QC], f16, tag="lns", name=f"lns{qc}_{h}")
            nc.scalar.activation(
                out=lns[:], in_=pv[DK : DK + 1, :], func=ln_fn
            )
            pending.append((pv, hm, hp, qc, lns))

        def attn(qc, fillers, pre=(), defer_last_quarter=False):
            for h in range(HPC):
                # h0's quarter reads AT rows written by this head's flush,
                # so it must trail the PVs; later heads' quarters are a
                # q-chunk old and slot in before the PVs as extra runway
                mid = None
                if qc > 0 and 0 < h and not (defer_last_quarter and h == HPC - 1):
                    mid = lambda h=h: outproj_quarter(qc - 1, h)
                attn_head(
                    qc,
                    h,
                    fillers[h] if h < len(fillers) else None,
                    interleave=pre if h == 0 else (),
                    mid=mid,
                )
                if qc > 0 and h == 0:
                    outproj_quarter(qc - 1, 0)
            if qc > 0 and defer_last_quarter:
                # always-ready PE work covering the last head's ln(sum)
                # latency before the final flush
                outproj_quarter(qc - 1, HPC - 1)

        def qk_fillers(n):
            return [
                lambda m=m, xt=xt, wt=wt, dst=dst, nm=nm: proj_qk_half(
                    xt, wt, dst, n, m, nm
                )
                for xt, wt, dst, nm in ((xq_t, wq_t, QT, "q"), (xk_t, wk_t, KT, "k"))
                for m in range(2)
            ]

        proj_qk(xq_t, wq_t, QT, 0, "q")
        proj_qk(xk_t, wk_t, KT, 0, "k")
        for n in range(nqc):
            pre = [
                (lambda st=st: proj_v_tile(st))
                for st in range(4 * n, 4 * n + 4)
            ]
            attn(
                n,
                qk_fillers(n + 1) if n + 1 < nqc else [],
                pre=pre,
                defer_last_quarter=(n + 1 == nqc),
            )
        flush_pending()
        # attention PSUM pools are done; reuse their banks for a deeper
        # tail pool so the final output projection streams without
        # rotation stalls
        pv_ps_cm.__exit__(None, None, None)
        st_ps_cm.__exit__(None, None, None)
        with tc.tile_pool(name="tailp", bufs=4, space="PSUM") as tailp:
            # dependency-free burst hides the last head's normalize (DVE
            # mul) latency before the tail projection reads AT
            wup2 = tailp.tile([P, QC], f32, tag="fp", name="tailwarm")
            for _ in range(8):
                nc.tensor.matmul(
                    wup2[:],
                    lhsT=dummy[:, 0:P],
                    rhs=dummy[:],
                    start=True,
                    stop=True,
                )
            for q in range(4):
                outproj_quarter(nqc - 1, q, copy_on_scalar=True, pool=tailp)

    return nc


def _build(mask_mode, seq=S):
    """v1 builder kept for the 'none'/'full' mask fallbacks."""
    import concourse.bass as bass
    import concourse.tile as tile
    from concourse import mybir
    from contextlib import ExitStack

    f32 = mybir.dt.float32
    bf16 = mybir.dt.bfloat16
    nqc = seq // QC
    nkt = seq // P
    nd = D // P  # 8 d-chunks

    nc = bass.Bass(num_swdge_queues=4)
    xq_d = nc.dram_tensor("xq_t", [D, seq], bf16, kind="ExternalInput")
    xk_d = nc.dram_tensor("xk_t", [D, seq], bf16, kind="ExternalInput")
    xv_d = nc.dram_tensor("xv_t", [D, seq], bf16, kind="ExternalInput")
    wq_d = nc.dram_tensor("wq_p", [P, D * DKB // P], bf16, kind="ExternalInput")
    wk_d = nc.dram_tensor("wk_p", [P, D * DKB // P], bf16, kind="ExternalInput")
    wv_d = nc.dram_tensor("wv_p", [P, D * DKB // P], bf16, kind="ExternalInput")
    wo_d = nc.dram_tensor("wo_p", [P, DKB * D // P], bf16, kind="ExternalInput")
    if mask_mode == "causal":
        stair_d = nc.dram_tensor("stair", [P, QC + 384], bf16, kind="ExternalInput")
    if mask_mode == "full":
        maskt_d = nc.dram_tensor("mask_t", [seq, seq], bf16, kind="ExternalInput")
    out_d = nc.dram_tensor("out", [seq, D], f32, kind="ExternalOutput")

    with ExitStack() as ctx:
        tc = ctx.enter_context(tile.TileContext(nc))
        persist = ctx.enter_context(tc.tile_pool(name="persist", bufs=1))

        ones64 = persist.tile([1, DK], f32, tag="ones64")
        nc.vector.memset(ones64[:], 1.0)
        wq_t = persist.tile([P, D * DKB // P], bf16, tag="wq")
        wk_t = persist.tile([P, D * DKB // P], bf16, tag="wk")
        wv_t = persist.tile([P, D * DKB // P], bf16, tag="wv")
        wo_t = persist.tile([P, DKB * D // P], bf16, tag="wo")
        nc.gpsimd.dma_start(out=wq_t[:], in_=wq_d[:, :])
        nc.gpsimd.dma_start(out=wk_t[:], in_=wk_d[:, :])
        nc.gpsimd.dma_start(out=wv_t[:], in_=wv_d[:, :])
        nc.gpsimd.dma_start(out=wo_t[:], in_=wo_d[:, :])
        if mask_mode == "causal":
            stair_t = persist.tile([P, QC + 384], bf16, tag="stair")
            nc.gpsimd.dma_start(out=stair_t[:], in_=stair_d[:, :])

        QT, KT, vt = [], [], []
        for m in range(2):
            QT.append(persist.tile([P, seq], bf16, tag=f"qt{m}", name=f"qt{m}"))
            KT.append(persist.tile([P, seq], bf16, tag=f"kt{m}", name=f"kt{m}"))
        AT = []
        for m in range(2):
            AT.append(persist.tile([P, seq], bf16, tag=f"at{m}", name=f"at{m}"))

        # ---- phase 1: projections (own PSUM + x pools, released after) ----
        with tc.tile_pool(name="xpool", bufs=1) as xpool, tc.tile_pool(
            name="projp", bufs=2, space="PSUM"
        ) as projp:

            def load_xt(xdram, name):
                t = xpool.tile([P, nd, seq], bf16, tag=name, name=name)
                h = nd // 2
                nc.sync.dma_start(
                    out=t[:, 0:h, :],
                    in_=xdram[: h * P, :].rearrange("(j p) s -> p j s", p=P),
                )
                nc.sync.dma_start(
                    out=t[:, h:nd, :],
                    in_=xdram[h * P :, :].rearrange("(j p) s -> p j s", p=P),
                )
                return t

            xq_t = load_xt(xq_d, "xq")
            xk_t = load_xt(xk_d, "xk")
            xv_t = load_xt(xv_d, "xv")

            def project_T(xt, wtile, res, name):
                ngroups = [
                    list(range(i, min(i + 2, nqc))) for i in range(0, nqc, 2)
                ]
                for m in range(2):
                    for gi, grp in enumerate(ngroups):
                        ps = projp.tile(
                            [P, len(grp) * QC],
                            f32,
                            tag="pj",
                            name=f"ps_{name}{m}_{gi}",
                        )
                        for half, n in enumerate(grp):
                            for j in range(nd):
                                nc.tensor.matmul(
                                    ps[:, half * QC : (half + 1) * QC],
                                    lhsT=wtile[
                                        :, j * DKB + m * P : j * DKB + (m + 1) * P
                                    ],
                                    rhs=xt[:, j, n * QC : (n + 1) * QC],
                                    start=(j == 0),
                                    stop=(j == nd - 1),
                                )
                        nc.vector.tensor_copy(
                            out=res[m][:, grp[0] * QC : (grp[-1] + 1) * QC],
                            in_=ps[:],
                        )

            project_T(xq_t, wq_t, QT, "qt")
            project_T(xk_t, wk_t, KT, "kt")

            # V natural layout [s, dv] + ones column per head
            for st in range(nkt):
                ps = projp.tile([P, DKB], f32, tag="pj", name=f"ps_v{st}")
                for j in range(nd):
                    nc.tensor.matmul(
                        ps[:],
                        lhsT=xv_t[:, j, st * P : (st + 1) * P],
                        rhs=wv_t[:, j * DKB : (j + 1) * DKB],
                        start=(j == 0),
                        stop=(j == nd - 1),
                    )
                v = persist.tile(
                    [P, HPC * (DK + 1)], bf16, tag=f"v{st}", name=f"v{st}"
                )
                nc.vector.memset(v[:], 1.0)
                nc.vector.tensor_copy(
                    out=v[:].rearrange("p (h w) -> p h w", w=DK + 1)[:, :, 0:DK],
                    in_=ps[:].rearrange("p (h w) -> p h w", w=DK),
                )
                vt.append(v)

        # ---- phase 2: attention (+ per-qc output projection) ----
        st_ps = ctx.enter_context(tc.tile_pool(name="st_ps", bufs=4, space="PSUM"))
        pv_ps = ctx.enter_context(tc.tile_pool(name="pv_ps", bufs=2, space="PSUM"))
        fp_ps = ctx.enter_context(tc.tile_pool(name="fp_ps", bufs=2, space="PSUM"))
        sc_pool = ctx.enter_context(tc.tile_pool(name="sc_pool", bufs=8))
        attn_pool = ctx.enter_context(tc.tile_pool(name="attn_pool", bufs=8))
        small = ctx.enter_context(tc.tile_pool(name="small", bufs=2))
        outp = ctx.enter_context(tc.tile_pool(name="outp", bufs=2))
        maskp = None
        if mask_mode == "full":
            maskp = ctx.enter_context(tc.tile_pool(name="maskp", bufs=2))

        exp_fn = mybir.ActivationFunctionType.Exp
        ln_fn = mybir.ActivationFunctionType.Ln
        for qc in range(nqc):
            mt = None
            if mask_mode == "full":
                mt = maskp.tile([P, nkt, QC], bf16, tag="mask", name=f"mt{qc}")
                nc.gpsimd.dma_start(
                    out=mt[:],
                    in_=maskt_d[:, qc * QC : (qc + 1) * QC].rearrange(
                        "(kt p) c -> p kt c", p=P
                    ),
                )
            for h in range(HPC):
                hm, hp = divmod(h, 2)
                hp *= DK
                if mask_mode == "causal":
                    kts = list(range(min(nkt, (qc + 1) * (QC // P))))
                else:
                    kts = list(range(nkt))
                pairs = [kts[i : i + 2] for i in range(0, len(kts), 2)]

                def trim(kt):
                    if mask_mode != "causal":
                        return 0, False
                    o = kt * P - qc * QC
                    return (o, True) if o >= 0 else (0, False)

                pv = pv_ps.tile([DK + 1, QC], f32, tag="pv", name=f"pv{qc}_{h}")

                def emit_pv(at, pair, is_last):
                    for half, kt in enumerate(pair):
                        o, _ = trim(kt)
                        nc.tensor.matmul(
                            pv[:, o:QC],
                            lhsT=vt[kt][:, h * (DK + 1) : (h + 1) * (DK + 1)],
                            rhs=at[:, half * QC + o : (half + 1) * QC],
                            start=(kt == 0),
                            stop=(is_last and half == len(pair) - 1),
                            skip_group_check=True,
                        )

                ats = []
                for pi, pair in enumerate(pairs):
                    sc = sc_pool.tile(
                        [P, 2 * QC], f32, tag="sc", name=f"sc{qc}_{h}_{pi}"
                    )
                    for half, kt in enumerate(pair):
                        o, diag = trim(kt)
                        stt = st_ps.tile(
                            [P, QC], f32, tag="st", name=f"st{qc}_{h}_{kt}"
                        )
                        nc.tensor.matmul(
                            stt[:, o:QC],
                            lhsT=KT[hm][hp : hp + DK, kt * P : (kt + 1) * P],
                            rhs=QT[hm][
                                hp : hp + DK, qc * QC + o : (qc + 1) * QC
                            ],
                            start=True,
                            stop=True,
                        )
                        dst = sc[:, half * QC + o : (half + 1) * QC]
                        if diag:
                            nc.vector.tensor_add(
                                out=dst,
                                in0=stt[:, o:QC],
                                in1=stair_t[:, 384 : 384 + QC - o],
                            )
                        elif mask_mode == "full":
                            nc.vector.tensor_add(
                                out=dst, in0=stt[:, o:QC], in1=mt[:, kt, :]
                            )
                        else:
                            nc.vector.tensor_copy(out=dst, in_=stt[:, o:QC])
                    at = attn_pool.tile(
                        [P, 2 * QC], bf16, tag="attn", name=f"a{qc}_{h}_{pi}"
                    )
                    o0, _ = trim(pair[0])
                    o1 = trim(pair[1])[0] if len(pair) > 1 else 0
                    if o1 == 0:
                        spans = [(o0, len(pair) * QC)]
                    else:
                        spans = [(o0, QC), (QC + o1, 2 * QC)]
                    for lo, hi in spans:
                        nc.scalar.activation(
                            out=at[:, lo:hi],
                            in_=sc[:, lo:hi],
                            func=exp_fn,
                            scale=0.125,
                        )
                    ats.append((at, pair))
                for at, pair in ats:
                    emit_pv(at, pair, pair is pairs[-1])
                # normalize: 1/sums = exp(-ln(sums)), broadcast via PE
                lns = small.tile([1, QC], f32, tag="lns", name=f"ln{qc}_{h}")
                nc.scalar.activation(
                    out=lns[:], in_=pv[DK : DK + 1, :], func=ln_fn
                )
                bcp = fp_ps.tile([DK, QC], f32, tag="fp", name=f"bcp{qc}_{h}")
                nc.tensor.matmul(
                    bcp[:], lhsT=ones64[:], rhs=lns[:], start=True, stop=True
                )
                bc = small.tile([DK, QC], f32, tag="bcast", name=f"bc{qc}_{h}")
                nc.scalar.activation(
                    out=bc[:], in_=bcp[:], func=exp_fn, scale=-1.0
                )
                nc.vector.tensor_mul(
                    AT[hm][hp : hp + DK, qc * QC : (qc + 1) * QC],
                    pv[0:DK, :],
                    bc[:],
                )

            # ---- output projection for this qc's s-tiles (keeps PE on
            # full-array matmuls between attention chunks; removes tail) ----
            for j2 in range(qc * (QC // (2 * P)), (qc + 1) * (QC // (2 * P))):
                ob = outp.tile([P, 2, D], f32, tag="ob", name=f"ob{j2}")
                for g in range(2):
                    st = 2 * j2 + g
                    for nch in range(D // QC):
                        ps = fp_ps.tile(
                            [P, QC], f32, tag="fp", name=f"ps_o{st}_{nch}"
                        )
                        for m in range(2):
                            nc.tensor.matmul(
                                ps[:],
                                lhsT=AT[m][:, st * P : (st + 1) * P],
                                rhs=wo_t[
                                    :, m * D + nch * QC : m * D + (nch + 1) * QC
                                ],
                                start=(m == 0),
                                stop=(m == 1),
                            )
                        nc.vector.tensor_copy(
                            out=ob[:, g, nch * QC : (nch + 1) * QC], in_=ps[:]
                        )
                nc.sync.dma_start(
                    out=out_d[j2 * 2 * P : (j2 + 1) * 2 * P, :].rearrange(
                        "(g p) n -> p g n", p=P
                    ),
                    in_=ob[:],
                )

    return nc


def _split_multi_waits(nc):
    """This toolchain's walrus accepts at most one sync-wait per
    instruction. Hoist extra waits onto preceding same-engine NoOps —
    engine streams execute in order, so a NoOp that blocks on a
    semaphore gates everything after it (including HWDGE descriptor
    enqueues, which happen when the issuing engine's sequencer reaches
    the DMA instruction)."""
    import bass_rust

    ctr = 0
    for f in nc.m.functions:
        for bb in f.blocks:
            insts = bb.instructions
            new = []
            changed = False
            for inst in insts:
                si = inst.sync_info
                if si is not None and len(si.on_wait) > 1:
                    waits = list(si.on_wait)
                    for w in waits[:-1]:
                        ctr += 1
                        nop = bass_rust.InstNoOp(
                            name=f"wsplit_{ctr}", ins=[], outs=[]
                        )
                        nop.engine = inst.engine
                        nop.sync_info = bass_rust.SyncInfo(
                            on_wait=[w], on_update=[]
                        )
                        new.append(nop)
                    inst.sync_info = bass_rust.SyncInfo(
                        on_wait=[waits[-1]], on_update=list(si.on_update)
                    )
                    changed = True
                new.append(inst)
            if changed:
                try:
                    bb.instructions = new
                except AttributeError:
                    insts.clear()
                    insts.extend(new)
    return nc


def _get_nc(mask_mode, seq=S, split_waits=True):
    key = (mask_mode, seq, split_waits)
    if key not in _nc_cache:
        if mask_mode == "causal":
            nc = _build_causal(seq)
        else:
            nc = _build(mask_mode, seq)
        if split_waits:
            _split_multi_waits(nc)
        _nc_cache[key] = nc
    return _nc_cache[key]


def _pack_w(w_slice_T, ncols):
    # [D_in, ncols] -> [128, D_in/128 * ncols]: col block j holds rows j*128..
    d_in = w_slice_T.shape[0]
    return (
        w_slice_T.reshape(d_in // P, P, ncols).transpose(1, 0, 2).reshape(P, -1)
    )


def _stair_np():
    cc = np.arange(QC + 384)[None, :]
    r = np.arange(P)[:, None]
    return np.where(cc < r + 384, np.float32(NEG), np.float32(0.0)).astype(BF16)


def _tri_np():
    # 0 where col < row (strictly below the shifted diagonal = masked)
    c = np.arange(P)[None, :]
    r = np.arange(P)[:, None]
    return np.where(c < r, np.float32(0.0), np.float32(1.0)).astype(BF16)


def _detect_mask_mode(mask):
    if not mask.any():
        return "none"
    causal = np.triu(np.ones((mask.shape[1], mask.shape[2]), bool), k=1)
    if all(np.array_equal(mask[b], causal) for b in range(mask.shape[0])):
        return "causal"
    return "full"


def _make_in_maps(query, key, value, mask, w_q, w_k, w_v, w_o, mask_mode, seq=S):
    per_batch = []
    for b in range(B):
        d = {
            "xq_t": np.ascontiguousarray(query[b].T).astype(BF16),
            "xk_t": np.ascontiguousarray(key[b].T).astype(BF16),
            "xv_t": np.ascontiguousarray(value[b].T).astype(BF16),
        }
        if mask_mode == "full":
            d["mask_t"] = np.where(
                mask[b].T, np.float32(NEG), np.float32(0.0)
            ).astype(BF16)
        per_batch.append(d)
    per_hg = []
    for hg in range(HGROUPS):
        rows = slice(hg * DKB, (hg + 1) * DKB)
        per_hg.append(
            {
                "wq_p": _pack_w(w_q[rows, :].T.astype(BF16), DKB),
                "wk_p": _pack_w(w_k[rows, :].T.astype(BF16), DKB),
                "wv_p": _pack_w(w_v[rows, :].T.astype(BF16), DKB),
                "wo_p": _pack_w(w_o[:, rows].T.astype(BF16), D),
            }
        )
    if mask_mode == "causal":
        extra = {"tri01": _tri_np()}
    elif mask_mode == "none":
        extra = {}
    else:
        extra = {}
    in_maps = []
    for c in range(NCORE):
        b, hg = divmod(c, HGROUPS)
        im = dict(per_batch[b])
        im.update(per_hg[hg])
        im.update(extra)
        in_maps.append(im)
    return in_maps


def _run(inputs, trace=False):
    from concourse.bass_utils import run_bass_kernel_spmd

    query = np.asarray(inputs["query"], np.float32)
    key = np.asarray(inputs["key"], np.float32)
    value = np.asarray(inputs["value"], np.float32)
    mask = np.asarray(inputs["mask"], bool)
    w_q = np.asarray(inputs["w_q"], np.float32)
    w_k = np.asarray(inputs["w_k"], np.float32)
    w_v = np.asarray(inputs["w_v"], np.float32)
    w_o = np.asarray(inputs["w_o"], np.float32)
    b_o = np.asarray(inputs["b_o"], np.float32)
    assert query.shape == (B, S, D), query.shape

    mask_mode = _detect_mask_mode(mask)
    nc = _get_nc(mask_mode)
    in_maps = _make_in_maps(query, key, value, mask, w_q, w_k, w_v, w_o, mask_mode)
    res = run_bass_kernel_spmd(nc, in_maps, list(range(NCORE)), trace=trace)
    outs = [np.asarray(r["out"], np.float32) for r in res.results]
    full = np.empty((B, S, D), np.float32)
    for b in range(B):
        full[b] = outs[HGROUPS * b]
        for i in range(1, HGROUPS):
            full[b] += outs[HGROUPS * b + i]
    full += b_o[None, None, :]
    return full, res


def kernel(**inputs):
    out, _ = _run(inputs, trace=False)
    return out


if __name__ == "__main__":
    import tempfile
    from concourse.bass_utils import compile_bass_kernel

    mode = sys.argv[1] if len(sys.argv) > 1 else "causal"
    nc = _get_nc(mode)
    from collections import Counter

    c = Counter()
    for name, inst in nc.inst_map.items():
        if "DMACopy" in type(inst).__name__:
            c[str(inst).count("wait:")] += 1
    print("DMA wait dist:", dict(c))
    td = tempfile.mkdtemp()
    p = compile_bass_kernel(nc, td)
    print("COMPILED OK:", p)
